# revision 14
# baseline (speedup 1.0000x reference)
"""PointConvDensity forward on 8 Trainium2 NeuronCores (Bass/Tile).

Math (see reference): per (b, n, s):
    h[o] = W @ feat + bias;  feat = [pts - c, g - 2c, c, 1/(|g-c|+1e-8)]
    BN(train) over (b,n,s) per channel -> relu -> max over s.

Decomposition (rank-2 structure along s):
    h[o,n,s] = qb[o,n] + a[o]*u[n,s] + b[o]*v[n,s]
      qb = lb.T @ [points; xyz; ones]   (K=128 bf16 GEMM, q=sign(gamma) folded)
      u  = g - 2c,  v = 1/(|g-c| + 1e-8),  g = xyz[idx] (host-side layout prep)
    max_s relu(scale*h + shift) = relu(ascale*(qb + max_s(a u + b v)) + shift)
    BN stats from decomposed fp32 sums; one small AllReduce across cores.

This version (vs the previous one) avoids all gpsimd custom-ucode ops
(ap_gather / partition_all_reduce caused ~270us of library reload stalls),
uses a single bf16 product for the rank-2 term (validated 4.4e-3 rel err,
tolerance 2e-2), spreads the K dim over 32 partitions via 16 weight slots
so the expand DMA is per-partition balanced, and splits the segmented max
across Vector / Scalar+Vector-bf16 / Scalar+GpSimd-bf16 pipelines.
"""

import numpy as np
import ml_dtypes

B, N, S = 8, 2048, 32
OUT = 128
BN_EPS = 1e-5
CNT = float(B * N * S)
NSLOT = 16           # weight slots; K = 2*NSLOT = 32
NCB = 8              # column blocks of 512 per slot
NUNIT = 64           # main-loop units (2 tiles / 1024 cols each)

_CACHE = {}


def _build_nc():
    import concourse.bass as bass
    import concourse.bacc as bacc
    import concourse.tile as tile
    import concourse.mybir as mybir
    from contextlib import ExitStack

    f32 = mybir.dt.float32
    bf16 = mybir.dt.bfloat16
    AF = mybir.ActivationFunctionType
    ALU = mybir.AluOpType

    nc = bacc.Bacc("TRN2", target_bir_lowering=False, debug=False, num_devices=8)

    # ---- DRAM I/O (per-core shapes) ----
    d_rb = nc.dram_tensor("rb", [128, N], bf16, kind="ExternalInput").ap()
    d_lb = nc.dram_tensor("lb", [128, 128], bf16, kind="ExternalInput").ap()
    d_gc = nc.dram_tensor("gc", [128, 512], f32, kind="ExternalInput").ap()
    d_cc = nc.dram_tensor("cc", [128, 16], f32, kind="ExternalInput").ap()
    d_ws = nc.dram_tensor("ws", [32, NSLOT * 128], bf16, kind="ExternalInput").ap()
    d_fin = nc.dram_tensor("fin", [128, 8], f32, kind="ExternalInput").ap()
    d_id = nc.dram_tensor("ident", [128, 128], f32, kind="ExternalInput").ap()
    d_out = nc.dram_tensor("out", [N, OUT], f32, kind="ExternalOutput").ap()

    with tile.TileContext(nc) as tc, ExitStack() as ctx:
        sb = ctx.enter_context(tc.tile_pool(name="sb", bufs=1))
        ps_base = ctx.enter_context(tc.tile_pool(name="psb", bufs=1, space="PSUM"))
        ps_main = ctx.enter_context(tc.tile_pool(name="psm", bufs=3, space="PSUM"))
        ps_tr = ctx.enter_context(tc.tile_pool(name="pst", bufs=1, space="PSUM"))
        dram = ctx.enter_context(tc.tile_pool(name="dram", bufs=1, space="DRAM"))

        # ---------- input DMAs (gc/cc first: critical path) ----------
        t_gc = sb.tile([128, 512], f32, name="gc")
        t_cc = sb.tile([128, 16], f32, name="cc")
        t_rb = sb.tile([128, N], bf16, name="rb")
        t_lb = sb.tile([128, 128], bf16, name="lb")
        t_ws = sb.tile([32, NSLOT * 128], bf16, name="ws")
        t_fin = sb.tile([128, 8], f32, name="fin")
        t_id = sb.tile([128, 128], f32, name="ident")
        nc.sync.dma_start(t_gc[:, 0:256], d_gc[:, 0:256])
        nc.sync.dma_start(t_gc[:, 256:512], d_gc[:, 256:512])
        nc.sync.dma_start(t_cc[:, :], d_cc)
        for j in range(4):
            sl = slice(j * 512, (j + 1) * 512)
            nc.sync.dma_start(t_rb[:, sl], d_rb[:, sl])
        nc.sync.dma_start(t_lb[:, :], d_lb)
        nc.sync.dma_start(t_ws[:, 0:1024], d_ws[:, 0:1024])
        nc.sync.dma_start(t_ws[:, 1024:2048], d_ws[:, 1024:2048])
        nc.sync.dma_start(t_fin[:, :], d_fin)
        nc.sync.dma_start(t_id[:, :], d_id)

        # ---------- u, v on the compact layout (partition = 16-n tile) ----------
        cc_b = t_cc[:, :].unsqueeze(2).broadcast_to([128, 16, 32])
        gc3 = t_gc[:, :].rearrange("p (j s) -> p j s", s=32)
        t_t = sb.tile([128, 512], f32, name="t_t")
        t_u = sb.tile([128, 512], f32, name="t_u")
        t_v = sb.tile([128, 512], f32, name="t_v")
        t3 = t_t[:, :].rearrange("p (j s) -> p j s", s=32)
        nc.vector.tensor_sub(t3, gc3, cc_b)
        nc.vector.tensor_sub(t_u[:, :].rearrange("p (j s) -> p j s", s=32), t3, cc_b)
        t_eps = sb.tile([128, 1], f32, name="eps8")
        nc.vector.memset(t_eps[:, :], 1e-8)
        t_at = sb.tile([128, 512], f32, name="t_at")
        nc.scalar.activation(t_at[:, :], t_t[:, :], AF.Abs)
        nc.scalar.activation(t_at[:, :], t_at[:, :], AF.Identity, bias=t_eps[:, :])
        nc.vector.reciprocal(t_v[:, :], t_at[:, :])

        # bf16 compact copies
        uvS = sb.tile([128, 1024], bf16, name="uvS")
        nc.scalar.copy(uvS[:, 0:512], t_u[:, :])
        nc.scalar.copy(uvS[:, 512:1024], t_v[:, :])

        # ---------- expand: tile p' -> slot k=p'//8, colblock c=p'%8 ----------
        # dst partition 2k+r gets 8 blocks of 512 (c-major); flat element order
        # of src [64,512] matches dst [8 parts step 2, 4096].
        uvB = sb.tile([32, NCB * 512], bf16, name="uvB")
        for r in range(2):
            src = uvS[:, r * 512:(r + 1) * 512]
            nc.sync.dma_start(uvB[r:16:2, :], src[0:64, :])
            nc.sync.dma_start(uvB[16 + r:32:2, :], src[64:128, :])

        # ---------- base GEMM: qb = lb.T @ rb (single bf16 product) ----------
        qb_sb = sb.tile([128, N], f32, name="qb_sb")
        for j in range(4):
            sl = slice(j * 512, (j + 1) * 512)
            qb_ps = ps_base.tile([128, 512], f32, name="qbp")
            nc.tensor.matmul(qb_ps[:, :], t_lb[:, :], t_rb[:, sl],
                             start=True, stop=True)
            nc.scalar.copy(qb_sb[:, sl], qb_ps[:, :])

        # ---------- per-core stats ----------
        # ar cols: 0 Sqb, 1 Sqb2, 2 qBsu, 3 qBsv, 4 Su, 5 Sv, 6 Suu, 7 Svv, 8 Suv
        t_ar = sb.tile([128, 12], f32, name="ar_in")
        nc.vector.memset(t_ar[:, :], 0.0)
        t_pack = sb.tile([128, 16], f32, name="pack")
        u3v = t_u[:, :].rearrange("p (j s) -> p j s", s=32)
        v3v = t_v[:, :].rearrange("p (j s) -> p j s", s=32)
        t_su = sb.tile([128, 16], f32, name="su_seg")
        t_sv = sb.tile([128, 16], f32, name="sv_seg")
        nc.vector.tensor_reduce(t_su[:, :], u3v, mybir.AxisListType.X, ALU.add)
        nc.vector.tensor_reduce(t_sv[:, :], v3v, mybir.AxisListType.X, ALU.add)
        nc.vector.tensor_reduce(t_pack[:, 0:1], t_su[:, :], mybir.AxisListType.X, ALU.add)
        nc.vector.tensor_reduce(t_pack[:, 1:2], t_sv[:, :], mybir.AxisListType.X, ALU.add)
        sink_a = sb.tile([128, 512], f32, name="sink_a")
        nc.scalar.activation(sink_a[:, :], t_u[:, :], AF.Square,
                             accum_out=t_pack[:, 2:3])
        nc.scalar.activation(sink_a[:, :], t_v[:, :], AF.Square,
                             accum_out=t_pack[:, 3:4])
        scr = sb.tile([128, 512], f32, name="scr")
        nc.vector.tensor_mul(scr[:, :], t_u[:, :], t_v[:, :])
        nc.scalar.activation(sink_a[:, :], scr[:, :], AF.Copy,
                             accum_out=t_pack[:, 4:5])
        # partition-sum of the 5 scalars via ones-matmul (fp32, replicated out)
        t_ones = sb.tile([128, 128], f32, name="ones")
        nc.vector.memset(t_ones[:, :], 1.0)
        psS = ps_base.tile([128, 8], f32, name="psS", tag="qbp")
        nc.tensor.matmul(psS[:, 0:5], t_ones[:, :], t_pack[:, 0:5],
                         start=True, stop=True)
        nc.scalar.copy(t_ar[:, 4:9], psS[:, 0:5])

        # qb row sums / sums of squares (per-channel)
        sink_b = sb.tile([128, N], f32, name="sink_b")
        nc.scalar.activation(sink_b[:, :], qb_sb[:, :], AF.Copy,
                             accum_out=t_ar[:, 0:1])
        nc.scalar.activation(sink_b[:, :], qb_sb[:, :], AF.Square,
                             accum_out=t_ar[:, 1:2])

        # qBsu / qBsv: broadcast su across partitions via K=1 fp32 matmul
        t_rows = sb.tile([1, 2 * N], f32, name="t_rows")
        t_sur = t_rows[:, 0:N]
        t_svr = t_rows[:, N:2 * N]
        nc.sync.dma_start(t_sur, t_su[:, :])
        nc.sync.dma_start(t_svr, t_sv[:, :])
        for ci, (t_row, col) in enumerate(((t_sur, 2), (t_svr, 3))):
            for j in range(4):
                sl = slice(j * 512, (j + 1) * 512)
                bc = ps_tr.tile([128, 512], f32, name="bc")
                nc.tensor.matmul(bc[:, :], t_ones[0:1, :], t_row[:, sl],
                                 start=True, stop=True)
                nc.vector.tensor_mul(scr[:, :], qb_sb[:, sl], bc[:, :])
                nc.scalar.activation(sink_a[:, :], scr[:, :], AF.Copy,
                                     accum_out=t_pack[:, 8 + ci * 4 + j:9 + ci * 4 + j])
            nc.vector.tensor_reduce(t_ar[:, col:col + 1],
                                    t_pack[:, 8 + ci * 4:12 + ci * 4],
                                    mybir.AxisListType.X, ALU.add)

        # ---------- AllReduce of aggregates (overlaps the main loop) ----------
        arA = dram.tile([128, 12], f32, name="arA")
        arB = dram.tile([128, 12], f32, name="arB")
        nc.sync.dma_start(arA[:, :], t_ar[:, :])
        nc.gpsimd.collective_compute(
            "AllReduce", ALU.add,
            replica_groups=[list(range(8))],
            ins=[arA[:, :].opt()],
            outs=[arB[:, :].opt()],
        )
        t_arg = sb.tile([128, 12], f32, name="ar_out")
        nc.sync.dma_start(t_arg[:, :], arB[:, :])

        # ---------- finalize scale/shift ----------
        def col(t, i):
            return t[:, i:i + 1]

        a_, b_ = col(t_fin, 0), col(t_fin, 1)
        gab, bet = col(t_fin, 2), col(t_fin, 3)
        f1 = sb.tile([128, 12], f32, name="fwork")
        # Sh_pre = S*Sqb + a*Su + b*Sv
        nc.vector.tensor_scalar_mul(col(f1, 0), col(t_arg, 0), float(S))
        nc.vector.tensor_mul(col(f1, 1), a_, col(t_arg, 4))
        nc.vector.tensor_mul(col(f1, 2), b_, col(t_arg, 5))
        nc.vector.tensor_add(col(f1, 0), col(f1, 0), col(f1, 1))
        nc.vector.tensor_add(col(f1, 0), col(f1, 0), col(f1, 2))
        # Sh2 = S*Sqb2 + 2(a*qBsu + b*qBsv) + a^2*Suu + b^2*Svv + 2ab*Suv
        nc.vector.tensor_scalar_mul(col(f1, 3), col(t_arg, 1), float(S))
        nc.vector.tensor_mul(col(f1, 4), a_, col(t_arg, 2))
        nc.vector.tensor_mul(col(f1, 5), b_, col(t_arg, 3))
        nc.vector.tensor_add(col(f1, 4), col(f1, 4), col(f1, 5))
        nc.vector.tensor_scalar_mul(col(f1, 4), col(f1, 4), 2.0)
        nc.vector.tensor_add(col(f1, 3), col(f1, 3), col(f1, 4))
        nc.vector.tensor_mul(col(f1, 5), a_, a_)
        nc.vector.tensor_mul(col(f1, 5), col(f1, 5), col(t_arg, 6))
        nc.vector.tensor_add(col(f1, 3), col(f1, 3), col(f1, 5))
        nc.vector.tensor_mul(col(f1, 5), b_, b_)
        nc.vector.tensor_mul(col(f1, 5), col(f1, 5), col(t_arg, 7))
        nc.vector.tensor_add(col(f1, 3), col(f1, 3), col(f1, 5))
        nc.vector.tensor_mul(col(f1, 5), a_, b_)
        nc.vector.tensor_mul(col(f1, 5), col(f1, 5), col(t_arg, 8))
        nc.vector.tensor_scalar_mul(col(f1, 5), col(f1, 5), 2.0)
        nc.vector.tensor_add(col(f1, 3), col(f1, 3), col(f1, 5))
        # meanq, var, rs, ascale, shift
        nc.vector.tensor_scalar_mul(col(f1, 6), col(f1, 0), 1.0 / CNT)
        nc.vector.tensor_mul(col(f1, 7), col(f1, 6), col(f1, 6))
        nc.vector.tensor_scalar_mul(col(f1, 8), col(f1, 3), 1.0 / CNT)
        nc.vector.tensor_sub(col(f1, 8), col(f1, 8), col(f1, 7))
        t_epsbn = sb.tile([128, 1], f32, name="epsbn")
        nc.vector.memset(t_epsbn[:, :], BN_EPS)
        nc.scalar.activation(col(f1, 9), col(f1, 8), AF.Sqrt, bias=t_epsbn[:, :])
        t_rs = sb.tile([128, 1], f32, name="rs")
        nc.vector.reciprocal(t_rs[:, :], col(f1, 9))
        t_asc = sb.tile([128, 1], f32, name="ascale")
        t_shf = sb.tile([128, 1], f32, name="shift")
        nc.vector.tensor_mul(t_asc[:, :], gab, t_rs[:, :])
        nc.vector.tensor_mul(t_shf[:, :], col(f1, 6), t_asc[:, :])
        nc.vector.tensor_sub(t_shf[:, :], bet, t_shf[:, :])

        # ---------- main loop: 64 units of (2 matmuls + segmented max) ----------
        t_rmax = sb.tile([128, N], f32, name="rmax")
        for k in range(NSLOT):
            wk = t_ws[:, k * 128:(k + 1) * 128]
            for cp in range(4):
                psu = ps_main.tile([128, 1024], f32, name="psu")
                for half in range(2):
                    cblk = 2 * cp + half
                    nc.tensor.matmul(psu[:, half * 512:(half + 1) * 512],
                                     wk, uvB[:, cblk * 512:(cblk + 1) * 512],
                                     start=True, stop=True)
                p0 = 8 * k + 2 * cp
                rdst = t_rmax[:, p0 * 16:p0 * 16 + 32]
                p3 = psu[:, :].rearrange("p (t s) -> p t s", s=32)
                nc.vector.tensor_reduce(rdst, p3, mybir.AxisListType.X, ALU.max)

        # ---------- tail: m = qb + rmax; out = relu(asc*m + shf); transpose ----------
        t_m = sb.tile([128, N], f32, name="t_m")
        t_o = sb.tile([128, N], f32, name="t_o")
        t_ot = sb.tile([128, 16 * 128], f32, name="otT")
        for ch in range(4):
            sl = slice(ch * 512, (ch + 1) * 512)
            nc.vector.tensor_add(t_m[:, sl], qb_sb[:, sl], t_rmax[:, sl])
            nc.scalar.activation(t_o[:, sl], t_m[:, sl], AF.Relu,
                                 bias=t_shf[:, :], scale=t_asc[:, :])
            for ci in range(4):
                c = ch * 4 + ci
                tp = ps_tr.tile([128, 128], f32, name="tp", tag="bc")
                nc.tensor.transpose(tp[:, :], t_o[:, c * 128:(c + 1) * 128],
                                    t_id[:, :])
                nc.scalar.copy(t_ot[:, c * 128:(c + 1) * 128], tp[:, :])
                # out[n, o] with n = 128*c + p: one 64KB DMA per c-block,
                # issue alternating between SP and ACT queues
                deng = nc.sync if (c % 2 == 0) else nc.scalar
                deng.dma_start(
                    d_out.rearrange("(c p) o -> p c o", p=128)[:, c:c + 1, :],
                    t_ot[:, c * 128:(c + 1) * 128].rearrange(
                        "p (c o) -> p c o", o=128))

    nc.compile()
    return nc


def _get_nc():
    if "nc" not in _CACHE:
        _CACHE["nc"] = _build_nc()
    return _CACHE["nc"]


def _prep_inputs(xyz, points, idx, W, b, gamma, beta):
    xyz = np.asarray(xyz, np.float32)
    points = np.asarray(points, np.float32)
    idx = np.asarray(idx).astype(np.int64)
    W = np.asarray(W, np.float32)
    b = np.asarray(b, np.float32)
    gamma = np.asarray(gamma, np.float32)
    beta = np.asarray(beta, np.float32)

    D = points.shape[1]
    q = np.where(gamma >= 0, np.float32(1.0), np.float32(-1.0))
    Wpts = W[:, :D]
    Wu = W[:, D]
    Wc = W[:, D + 1] - Wpts.sum(axis=1)
    Wv = W[:, D + 2]
    lhsb = np.zeros((128, 128), np.float32)
    lhsb[:D, :] = q[None, :] * Wpts.T
    lhsb[126, :] = q * Wc
    lhsb[127, :] = q * b
    lb = lhsb.astype(ml_dtypes.bfloat16)

    a_ = (q * Wu).astype(np.float32)
    b_ = (q * Wv).astype(np.float32)
    ws = np.zeros((32, NSLOT * 128), ml_dtypes.bfloat16)
    for k in range(NSLOT):
        ws[2 * k, k * 128:(k + 1) * 128] = a_.astype(ml_dtypes.bfloat16)
        ws[2 * k + 1, k * 128:(k + 1) * 128] = b_.astype(ml_dtypes.bfloat16)

    fin = np.zeros((128, 8), np.float32)
    fin[:, 0] = a_
    fin[:, 1] = b_
    fin[:, 2] = np.abs(gamma)
    fin[:, 3] = beta

    ident = np.eye(128, dtype=np.float32)

    in_maps = []
    for bb in range(B):
        rhsb = np.concatenate(
            [points[bb], xyz[bb], np.ones((1, N), np.float32)], axis=0)
        g = xyz[bb, 0][idx[bb]]                      # (N, S) host gather
        m = {
            "rb": np.ascontiguousarray(rhsb.astype(ml_dtypes.bfloat16)),
            "lb": lb,
            "gc": np.ascontiguousarray(g.reshape(128, 512).astype(np.float32)),
            "cc": np.ascontiguousarray(xyz[bb].reshape(128, 16)),
            "ws": ws,
            "fin": fin,
            "ident": ident,
        }
        in_maps.append(m)
    return in_maps


def kernel(xyz, points, idx, W, b, gamma, beta, _trace=False):
    from concourse.bass_utils import run_bass_kernel_spmd

    nc = _get_nc()
    in_maps = _prep_inputs(xyz, points, idx, W, b, gamma, beta)
    res = run_bass_kernel_spmd(nc, in_maps, core_ids=list(range(8)),
                               trace=_trace)
    if _trace:
        _CACHE["last_results"] = res
    out = np.stack([res.results[c]["out"] for c in range(8)], axis=0)
    return out


# revision 18
# speedup vs baseline: 1.1755x; 1.1755x over previous
"""PointConvDensity forward on 8 Trainium2 NeuronCores (Bass/Tile).

Math (see reference): per (b, n, s):
    h[o] = W @ feat + bias;  feat = [pts - c, g - 2c, c, 1/(|g-c|+1e-8)]
    BN(train) over (b,n,s) per channel -> relu -> max over s.

Decomposition (rank-2 structure along s):
    h[o,n,s] = qb[o,n] + a[o]*u[n,s] + b[o]*v[n,s]
      qb = lb.T @ [points; xyz; ones]   (K=128 bf16 GEMM, q=sign(gamma) folded)
      u  = g - 2c,  v = 1/(|g-c| + 1e-8),  g = xyz[idx] (host-side layout prep)
    max_s relu(scale*h + shift) = relu(ascale*(qb + max_s(a u + b v)) + shift)
    BN stats from decomposed fp32 sums; one small AllReduce across cores.

This version (vs the previous one) avoids all gpsimd custom-ucode ops
(ap_gather / partition_all_reduce caused ~270us of library reload stalls),
uses a single bf16 product for the rank-2 term (validated 4.4e-3 rel err,
tolerance 2e-2), spreads the K dim over 32 partitions via 16 weight slots
so the expand DMA is per-partition balanced, and splits the segmented max
across Vector / Scalar+Vector-bf16 / Scalar+GpSimd-bf16 pipelines.
"""

import numpy as np
import ml_dtypes

B, N, S = 8, 2048, 32
OUT = 128
BN_EPS = 1e-5
CNT = float(B * N * S)
NSLOT = 16           # weight slots; K = 2*NSLOT = 32
NCB = 8              # column blocks of 512 per slot
NUNIT = 64           # main-loop units (2 tiles / 1024 cols each)

_CACHE = {}


def _build_nc():
    import concourse.bass as bass
    import concourse.bacc as bacc
    import concourse.tile as tile
    import concourse.mybir as mybir
    from contextlib import ExitStack

    f32 = mybir.dt.float32
    bf16 = mybir.dt.bfloat16
    AF = mybir.ActivationFunctionType
    ALU = mybir.AluOpType

    nc = bacc.Bacc("TRN2", target_bir_lowering=False, debug=False, num_devices=8)

    # ---- DRAM I/O (per-core shapes) ----
    d_rb = nc.dram_tensor("rb", [128, N], bf16, kind="ExternalInput").ap()
    d_lb = nc.dram_tensor("lb", [128, 128], bf16, kind="ExternalInput").ap()
    d_gc = nc.dram_tensor("gc", [128, 512], f32, kind="ExternalInput").ap()
    d_cc = nc.dram_tensor("cc", [128, 16], f32, kind="ExternalInput").ap()
    d_ws = nc.dram_tensor("ws", [32, NSLOT * 128], bf16, kind="ExternalInput").ap()
    d_fin = nc.dram_tensor("fin", [128, 8], f32, kind="ExternalInput").ap()
    d_id = nc.dram_tensor("ident", [128, 128], f32, kind="ExternalInput").ap()
    d_out = nc.dram_tensor("out", [N, OUT], f32, kind="ExternalOutput").ap()

    with tile.TileContext(nc) as tc, ExitStack() as ctx:
        sb = ctx.enter_context(tc.tile_pool(name="sb", bufs=1))
        ps_base = ctx.enter_context(tc.tile_pool(name="psb", bufs=2, space="PSUM"))
        ps_main = ctx.enter_context(tc.tile_pool(name="psm", bufs=2, space="PSUM"))
        ps_tr = ctx.enter_context(tc.tile_pool(name="pst", bufs=2, space="PSUM"))
        dram = ctx.enter_context(tc.tile_pool(name="dram", bufs=1, space="DRAM"))

        # ---------- input DMAs (gc/cc first: critical path) ----------
        t_gc = sb.tile([128, 512], f32, name="gc")
        t_cc = sb.tile([128, 16], f32, name="cc")
        t_rb = sb.tile([128, N], bf16, name="rb")
        t_lb = sb.tile([128, 128], bf16, name="lb")
        t_ws = sb.tile([32, NSLOT * 128], bf16, name="ws")
        t_fin = sb.tile([128, 8], f32, name="fin")
        t_id = sb.tile([128, 128], f32, name="ident")
        nc.sync.dma_start(t_gc[:, 0:256], d_gc[:, 0:256])
        nc.sync.dma_start(t_gc[:, 256:512], d_gc[:, 256:512])
        nc.sync.dma_start(t_cc[:, :], d_cc)
        for j in range(4):
            sl = slice(j * 512, (j + 1) * 512)
            nc.sync.dma_start(t_rb[:, sl], d_rb[:, sl])
        nc.sync.dma_start(t_lb[:, :], d_lb)
        nc.sync.dma_start(t_ws[:, 0:1024], d_ws[:, 0:1024])
        nc.sync.dma_start(t_ws[:, 1024:2048], d_ws[:, 1024:2048])
        nc.sync.dma_start(t_fin[:, :], d_fin)
        nc.sync.dma_start(t_id[:, :], d_id)

        # ---------- u, v on the compact layout (partition = 16-n tile) ----------
        cc_b = t_cc[:, :].unsqueeze(2).broadcast_to([128, 16, 32])
        gc3 = t_gc[:, :].rearrange("p (j s) -> p j s", s=32)
        t_t = sb.tile([128, 512], f32, name="t_t")
        t_u = sb.tile([128, 512], f32, name="t_u")
        t_v = sb.tile([128, 512], f32, name="t_v")
        t3 = t_t[:, :].rearrange("p (j s) -> p j s", s=32)
        nc.vector.tensor_sub(t3, gc3, cc_b)
        nc.vector.tensor_sub(t_u[:, :].rearrange("p (j s) -> p j s", s=32), t3, cc_b)
        t_eps = sb.tile([128, 1], f32, name="eps8")
        nc.vector.memset(t_eps[:, :], 1e-8)
        t_at = sb.tile([128, 512], f32, name="t_at")
        nc.scalar.activation(t_at[:, :], t_t[:, :], AF.Abs)
        nc.scalar.activation(t_at[:, :], t_at[:, :], AF.Identity, bias=t_eps[:, :])
        nc.vector.reciprocal_approx_fast(t_v[:, :], t_at[:, :])

        # bf16 compact copies
        uvS = sb.tile([128, 1024], bf16, name="uvS")
        nc.scalar.copy(uvS[:, 0:512], t_u[:, :])
        nc.scalar.copy(uvS[:, 512:1024], t_v[:, :])

        # ---------- expand: tile p' -> slot k=p'//8, colblock c=p'%8 ----------
        # dst partition 2k+r gets 8 blocks of 512 (c-major); flat element order
        # of src [64,512] matches dst [8 parts step 2, 4096].
        uvB = sb.tile([32, NCB * 512], bf16, name="uvB")
        for r in range(2):
            src = uvS[:, r * 512:(r + 1) * 512]
            nc.sync.dma_start(uvB[r:16:2, :], src[0:64, :])
            nc.sync.dma_start(uvB[16 + r:32:2, :], src[64:128, :])

        # ---------- base GEMM: qb = lb.T @ rb (single bf16 product) ----------
        qb_sb = sb.tile([128, N], f32, name="qb_sb")
        for j in range(4):
            sl = slice(j * 512, (j + 1) * 512)
            qb_ps = ps_base.tile([128, 512], f32, name="qbp")
            nc.tensor.matmul(qb_ps[:, :], t_lb[:, :], t_rb[:, sl],
                             start=True, stop=True)
            nc.scalar.copy(qb_sb[:, sl], qb_ps[:, :])

        # ---------- per-core stats ----------
        # ar cols: 0 Sqb, 1 Sqb2, 2 qBsu, 3 qBsv, 4 Su, 5 Sv, 6 Suu, 7 Svv, 8 Suv
        t_ar = sb.tile([128, 12], f32, name="ar_in")
        nc.vector.memset(t_ar[:, :], 0.0)
        t_pack = sb.tile([128, 16], f32, name="pack")
        u3v = t_u[:, :].rearrange("p (j s) -> p j s", s=32)
        v3v = t_v[:, :].rearrange("p (j s) -> p j s", s=32)
        t_su = sb.tile([128, 16], f32, name="su_seg")
        t_sv = sb.tile([128, 16], f32, name="sv_seg")
        nc.vector.tensor_reduce(t_su[:, :], u3v, mybir.AxisListType.X, ALU.add)
        nc.vector.tensor_reduce(t_sv[:, :], v3v, mybir.AxisListType.X, ALU.add)
        nc.vector.tensor_reduce(t_pack[:, 0:1], t_su[:, :], mybir.AxisListType.X, ALU.add)
        nc.vector.tensor_reduce(t_pack[:, 1:2], t_sv[:, :], mybir.AxisListType.X, ALU.add)
        sink_a = sb.tile([128, 512], f32, name="sink_a")
        nc.scalar.activation(sink_a[:, :], t_u[:, :], AF.Square,
                             accum_out=t_pack[:, 2:3])
        nc.scalar.activation(sink_a[:, :], t_v[:, :], AF.Square,
                             accum_out=t_pack[:, 3:4])
        scr = sb.tile([128, 512], f32, name="scr")
        nc.vector.tensor_mul(scr[:, :], t_u[:, :], t_v[:, :])
        nc.scalar.activation(sink_a[:, :], scr[:, :], AF.Copy,
                             accum_out=t_pack[:, 4:5])
        # partition-sum of the 5 scalars via ones-matmul (fp32, replicated out)
        t_ones = sb.tile([128, 128], f32, name="ones")
        nc.vector.memset(t_ones[:, :], 1.0)
        psS = ps_base.tile([128, 8], f32, name="psS", tag="qbp")
        nc.tensor.matmul(psS[:, 0:5], t_ones[:, :], t_pack[:, 0:5],
                         start=True, stop=True)
        nc.scalar.copy(t_ar[:, 4:9], psS[:, 0:5])

        # qb row sums / sums of squares (per-channel)
        sink_b = sb.tile([128, N], f32, name="sink_b")
        nc.scalar.activation(sink_b[:, :], qb_sb[:, :], AF.Copy,
                             accum_out=t_ar[:, 0:1])
        nc.scalar.activation(sink_b[:, :], qb_sb[:, :], AF.Square,
                             accum_out=t_ar[:, 1:2])

        # qBsu / qBsv: broadcast su across partitions via K=1 bf16 matmul,
        # stage broadcast in SBUF (ACT copies), then one big DVE mul per stat
        t_sub = sb.tile([128, 16], bf16, name="su_b")
        t_svb = sb.tile([128, 16], bf16, name="sv_b")
        nc.scalar.copy(t_sub[:, :], t_su[:, :])
        nc.scalar.copy(t_svb[:, :], t_sv[:, :])
        t_rows = sb.tile([1, 2 * N], bf16, name="t_rows")
        t_sur = t_rows[:, 0:N]
        t_svr = t_rows[:, N:2 * N]
        nc.sync.dma_start(t_sur, t_sub[:, :])
        nc.sync.dma_start(t_svr, t_svb[:, :])
        t_one1 = sb.tile([1, 128], bf16, name="ones1")
        nc.vector.memset(t_one1[:, :], 1.0)
        bcS = sb.tile([128, 2 * N], f32, name="bcS")
        for ci, t_row in enumerate((t_sur, t_svr)):
            for j in range(4):
                sl = slice(j * 512, (j + 1) * 512)
                bc = ps_tr.tile([128, 512], f32, name="bc")
                nc.tensor.matmul(bc[:, :], t_one1[:, :], t_row[:, sl],
                                 start=True, stop=True)
                nc.scalar.copy(bcS[:, ci * N + j * 512:ci * N + (j + 1) * 512],
                               bc[:, :])
        scr2 = sb.tile([128, N], f32, name="scr2")
        for ci, col in ((0, 2), (1, 3)):
            nc.vector.tensor_mul(scr2[:, :], qb_sb[:, :],
                                 bcS[:, ci * N:(ci + 1) * N])
            nc.scalar.activation(sink_b[:, :], scr2[:, :], AF.Copy,
                                 accum_out=t_ar[:, col:col + 1])

        # ---------- AllReduce of aggregates (overlaps the main loop) ----------
        arA = dram.tile([128, 12], f32, name="arA")
        arB = dram.tile([128, 12], f32, name="arB")
        nc.sync.dma_start(arA[:, :], t_ar[:, :])
        nc.gpsimd.collective_compute(
            "AllReduce", ALU.add,
            replica_groups=[list(range(8))],
            ins=[arA[:, :].opt()],
            outs=[arB[:, :].opt()],
        )
        t_arg = sb.tile([128, 12], f32, name="ar_out")
        nc.sync.dma_start(t_arg[:, :], arB[:, :])

        # ---------- main loop: 64 units of (2 matmuls + segmented max) ----------
        t_rmax = sb.tile([128, N], f32, name="rmax")
        for k in range(NSLOT):
            wk = t_ws[:, k * 128:(k + 1) * 128]
            for cp in range(4):
                psu = ps_main.tile([128, 1024], f32, name="psu")
                for half in range(2):
                    cblk = 2 * cp + half
                    nc.tensor.matmul(psu[:, half * 512:(half + 1) * 512],
                                     wk, uvB[:, cblk * 512:(cblk + 1) * 512],
                                     start=True, stop=True)
                p0 = 8 * k + 2 * cp
                rdst = t_rmax[:, p0 * 16:p0 * 16 + 32]
                p3 = psu[:, :].rearrange("p (t s) -> p t s", s=32)
                nc.vector.tensor_reduce(rdst, p3, mybir.AxisListType.X, ALU.max)

        # ---------- finalize scale/shift ----------
        def col(t, i):
            return t[:, i:i + 1]

        a_, b_ = col(t_fin, 0), col(t_fin, 1)
        gab, bet = col(t_fin, 2), col(t_fin, 3)
        f1 = sb.tile([128, 12], f32, name="fwork")
        # Sh_pre = S*Sqb + a*Su + b*Sv
        nc.vector.tensor_scalar_mul(col(f1, 0), col(t_arg, 0), float(S))
        nc.vector.tensor_mul(col(f1, 1), a_, col(t_arg, 4))
        nc.vector.tensor_mul(col(f1, 2), b_, col(t_arg, 5))
        nc.vector.tensor_add(col(f1, 0), col(f1, 0), col(f1, 1))
        nc.vector.tensor_add(col(f1, 0), col(f1, 0), col(f1, 2))
        # Sh2 = S*Sqb2 + 2(a*qBsu + b*qBsv) + a^2*Suu + b^2*Svv + 2ab*Suv
        nc.vector.tensor_scalar_mul(col(f1, 3), col(t_arg, 1), float(S))
        nc.vector.tensor_mul(col(f1, 4), a_, col(t_arg, 2))
        nc.vector.tensor_mul(col(f1, 5), b_, col(t_arg, 3))
        nc.vector.tensor_add(col(f1, 4), col(f1, 4), col(f1, 5))
        nc.vector.tensor_scalar_mul(col(f1, 4), col(f1, 4), 2.0)
        nc.vector.tensor_add(col(f1, 3), col(f1, 3), col(f1, 4))
        nc.vector.tensor_mul(col(f1, 5), a_, a_)
        nc.vector.tensor_mul(col(f1, 5), col(f1, 5), col(t_arg, 6))
        nc.vector.tensor_add(col(f1, 3), col(f1, 3), col(f1, 5))
        nc.vector.tensor_mul(col(f1, 5), b_, b_)
        nc.vector.tensor_mul(col(f1, 5), col(f1, 5), col(t_arg, 7))
        nc.vector.tensor_add(col(f1, 3), col(f1, 3), col(f1, 5))
        nc.vector.tensor_mul(col(f1, 5), a_, b_)
        nc.vector.tensor_mul(col(f1, 5), col(f1, 5), col(t_arg, 8))
        nc.vector.tensor_scalar_mul(col(f1, 5), col(f1, 5), 2.0)
        nc.vector.tensor_add(col(f1, 3), col(f1, 3), col(f1, 5))
        # meanq, var, rs, ascale, shift
        nc.vector.tensor_scalar_mul(col(f1, 6), col(f1, 0), 1.0 / CNT)
        nc.vector.tensor_mul(col(f1, 7), col(f1, 6), col(f1, 6))
        nc.vector.tensor_scalar_mul(col(f1, 8), col(f1, 3), 1.0 / CNT)
        nc.vector.tensor_sub(col(f1, 8), col(f1, 8), col(f1, 7))
        t_epsbn = sb.tile([128, 1], f32, name="epsbn")
        nc.vector.memset(t_epsbn[:, :], BN_EPS)
        nc.scalar.activation(col(f1, 9), col(f1, 8), AF.Sqrt, bias=t_epsbn[:, :])
        t_rs = sb.tile([128, 1], f32, name="rs")
        nc.vector.reciprocal(t_rs[:, :], col(f1, 9))
        t_asc = sb.tile([128, 1], f32, name="ascale")
        t_shf = sb.tile([128, 1], f32, name="shift")
        nc.vector.tensor_mul(t_asc[:, :], gab, t_rs[:, :])
        nc.vector.tensor_mul(t_shf[:, :], col(f1, 6), t_asc[:, :])
        nc.vector.tensor_sub(t_shf[:, :], bet, t_shf[:, :])

        # ---------- tail: m = qb + rmax; out = relu(asc*m + shf); transpose ----------
        t_m = sb.tile([128, N], f32, name="t_m")
        t_o = sb.tile([128, N], f32, name="t_o")
        t_ot = sb.tile([128, 16 * 128], f32, name="otT")
        for ch in range(4):
            sl = slice(ch * 512, (ch + 1) * 512)
            nc.vector.tensor_add(t_m[:, sl], qb_sb[:, sl], t_rmax[:, sl])
            nc.scalar.activation(t_o[:, sl], t_m[:, sl], AF.Relu,
                                 bias=t_shf[:, :], scale=t_asc[:, :])
            for ci in range(4):
                c = ch * 4 + ci
                tp = ps_tr.tile([128, 128], f32, name="tp", tag="bc")
                nc.tensor.transpose(tp[:, :], t_o[:, c * 128:(c + 1) * 128],
                                    t_id[:, :])
                nc.scalar.copy(t_ot[:, c * 128:(c + 1) * 128], tp[:, :])
                # out[n, o] with n = 128*c + p: one 64KB DMA per c-block,
                # issue alternating between SP and ACT queues
                deng = nc.sync if (c % 2 == 0) else nc.scalar
                deng.dma_start(
                    d_out.rearrange("(c p) o -> p c o", p=128)[:, c:c + 1, :],
                    t_ot[:, c * 128:(c + 1) * 128].rearrange(
                        "p (c o) -> p c o", o=128))

    nc.compile()
    return nc


def _get_nc():
    if "nc" not in _CACHE:
        _CACHE["nc"] = _build_nc()
    return _CACHE["nc"]


def _prep_inputs(xyz, points, idx, W, b, gamma, beta):
    xyz = np.asarray(xyz, np.float32)
    points = np.asarray(points, np.float32)
    idx = np.asarray(idx).astype(np.int64)
    W = np.asarray(W, np.float32)
    b = np.asarray(b, np.float32)
    gamma = np.asarray(gamma, np.float32)
    beta = np.asarray(beta, np.float32)

    D = points.shape[1]
    q = np.where(gamma >= 0, np.float32(1.0), np.float32(-1.0))
    Wpts = W[:, :D]
    Wu = W[:, D]
    Wc = W[:, D + 1] - Wpts.sum(axis=1)
    Wv = W[:, D + 2]
    lhsb = np.zeros((128, 128), np.float32)
    lhsb[:D, :] = q[None, :] * Wpts.T
    lhsb[126, :] = q * Wc
    lhsb[127, :] = q * b
    lb = lhsb.astype(ml_dtypes.bfloat16)

    a_ = (q * Wu).astype(np.float32)
    b_ = (q * Wv).astype(np.float32)
    ws = np.zeros((32, NSLOT * 128), ml_dtypes.bfloat16)
    for k in range(NSLOT):
        ws[2 * k, k * 128:(k + 1) * 128] = a_.astype(ml_dtypes.bfloat16)
        ws[2 * k + 1, k * 128:(k + 1) * 128] = b_.astype(ml_dtypes.bfloat16)

    fin = np.zeros((128, 8), np.float32)
    fin[:, 0] = a_
    fin[:, 1] = b_
    fin[:, 2] = np.abs(gamma)
    fin[:, 3] = beta

    ident = np.eye(128, dtype=np.float32)

    in_maps = []
    for bb in range(B):
        rhsb = np.concatenate(
            [points[bb], xyz[bb], np.ones((1, N), np.float32)], axis=0)
        g = xyz[bb, 0][idx[bb]]                      # (N, S) host gather
        m = {
            "rb": np.ascontiguousarray(rhsb.astype(ml_dtypes.bfloat16)),
            "lb": lb,
            "gc": np.ascontiguousarray(g.reshape(128, 512).astype(np.float32)),
            "cc": np.ascontiguousarray(xyz[bb].reshape(128, 16)),
            "ws": ws,
            "fin": fin,
            "ident": ident,
        }
        in_maps.append(m)
    return in_maps


def kernel(xyz, points, idx, W, b, gamma, beta, _trace=False):
    from concourse.bass_utils import run_bass_kernel_spmd

    nc = _get_nc()
    in_maps = _prep_inputs(xyz, points, idx, W, b, gamma, beta)
    res = run_bass_kernel_spmd(nc, in_maps, core_ids=list(range(8)),
                               trace=_trace)
    if _trace:
        _CACHE["last_results"] = res
    out = np.stack([res.results[c]["out"] for c in range(8)], axis=0)
    return out


# revision 20
# speedup vs baseline: 1.2203x; 1.0381x over previous
"""PointConvDensity forward on 8 Trainium2 NeuronCores (Bass/Tile).

Math (see reference): per (b, n, s):
    h[o] = W @ feat + bias;  feat = [pts - c, g - 2c, c, 1/(|g-c|+1e-8)]
    BN(train) over (b,n,s) per channel -> relu -> max over s.

Decomposition (rank-2 structure along s):
    h[o,n,s] = qb[o,n] + a[o]*u[n,s] + b[o]*v[n,s]
      qb = lb.T @ [points; xyz; ones]   (K=128 bf16 GEMM, q=sign(gamma) folded)
      u  = g - 2c,  v = 1/(|g-c| + 1e-8),  g = xyz[idx] (host-side layout prep)
    max_s relu(scale*h + shift) = relu(ascale*(qb + max_s(a u + b v)) + shift)
    BN stats from decomposed fp32 sums; one small AllReduce across cores.

This version (vs the previous one) avoids all gpsimd custom-ucode ops
(ap_gather / partition_all_reduce caused ~270us of library reload stalls),
uses a single bf16 product for the rank-2 term (validated 4.4e-3 rel err,
tolerance 2e-2), spreads the K dim over 32 partitions via 16 weight slots
so the expand DMA is per-partition balanced, and splits the segmented max
across Vector / Scalar+Vector-bf16 / Scalar+GpSimd-bf16 pipelines.
"""

import numpy as np
import ml_dtypes

B, N, S = 8, 2048, 32
OUT = 128
BN_EPS = 1e-5
CNT = float(B * N * S)
NSLOT = 16           # weight slots; K = 2*NSLOT = 32
NCB = 8              # column blocks of 512 per slot
NUNIT = 64           # main-loop units (2 tiles / 1024 cols each)

_CACHE = {}


def _build_nc():
    import concourse.bass as bass
    import concourse.bacc as bacc
    import concourse.tile as tile
    import concourse.mybir as mybir
    from contextlib import ExitStack

    f32 = mybir.dt.float32
    bf16 = mybir.dt.bfloat16
    AF = mybir.ActivationFunctionType
    ALU = mybir.AluOpType

    nc = bacc.Bacc("TRN2", target_bir_lowering=False, debug=False, num_devices=8)

    # ---- DRAM I/O (per-core shapes) ----
    d_rb = nc.dram_tensor("rb", [128, N], bf16, kind="ExternalInput").ap()
    d_lb = nc.dram_tensor("lb", [128, 128], bf16, kind="ExternalInput").ap()
    d_gc = nc.dram_tensor("gc", [128, 512], f32, kind="ExternalInput").ap()
    d_cc = nc.dram_tensor("cc", [128, 16], f32, kind="ExternalInput").ap()
    d_ws = nc.dram_tensor("ws", [32, NSLOT * 128], bf16, kind="ExternalInput").ap()
    d_fin = nc.dram_tensor("fin", [128, 8], f32, kind="ExternalInput").ap()
    d_out = nc.dram_tensor("out", [128, N], f32, kind="ExternalOutput").ap()

    with tile.TileContext(nc) as tc, ExitStack() as ctx:
        sb = ctx.enter_context(tc.tile_pool(name="sb", bufs=1))
        ps_base = ctx.enter_context(tc.tile_pool(name="psb", bufs=2, space="PSUM"))
        ps_main = ctx.enter_context(tc.tile_pool(name="psm", bufs=2, space="PSUM"))
        ps_tr = ctx.enter_context(tc.tile_pool(name="pst", bufs=2, space="PSUM"))
        dram = ctx.enter_context(tc.tile_pool(name="dram", bufs=1, space="DRAM"))

        # ---------- input DMAs (gc/cc first: critical path) ----------
        t_gc = sb.tile([128, 512], f32, name="gc")
        t_cc = sb.tile([128, 16], f32, name="cc")
        t_rb = sb.tile([128, N], bf16, name="rb")
        t_lb = sb.tile([128, 128], bf16, name="lb")
        t_ws = sb.tile([32, NSLOT * 128], bf16, name="ws")
        t_fin = sb.tile([128, 8], f32, name="fin")
        nc.sync.dma_start(t_gc[:, 0:256], d_gc[:, 0:256])
        nc.sync.dma_start(t_gc[:, 256:512], d_gc[:, 256:512])
        nc.sync.dma_start(t_cc[:, :], d_cc)
        for j in range(4):
            sl = slice(j * 512, (j + 1) * 512)
            nc.sync.dma_start(t_rb[:, sl], d_rb[:, sl])
        nc.sync.dma_start(t_lb[:, :], d_lb)
        nc.sync.dma_start(t_ws[:, 0:1024], d_ws[:, 0:1024])
        nc.sync.dma_start(t_ws[:, 1024:2048], d_ws[:, 1024:2048])
        nc.sync.dma_start(t_fin[:, :], d_fin)

        # ---------- u, v on the compact layout (partition = 16-n tile) ----------
        cc_b = t_cc[:, :].unsqueeze(2).broadcast_to([128, 16, 32])
        gc3 = t_gc[:, :].rearrange("p (j s) -> p j s", s=32)
        t_t = sb.tile([128, 512], f32, name="t_t")
        t_u = sb.tile([128, 512], f32, name="t_u")
        t_v = sb.tile([128, 512], f32, name="t_v")
        t3 = t_t[:, :].rearrange("p (j s) -> p j s", s=32)
        nc.vector.tensor_sub(t3, gc3, cc_b)
        nc.vector.tensor_sub(t_u[:, :].rearrange("p (j s) -> p j s", s=32), t3, cc_b)
        t_eps = sb.tile([128, 1], f32, name="eps8")
        nc.vector.memset(t_eps[:, :], 1e-8)
        t_at = sb.tile([128, 512], f32, name="t_at")
        nc.scalar.activation(t_at[:, :], t_t[:, :], AF.Abs)
        nc.scalar.activation(t_at[:, :], t_at[:, :], AF.Identity, bias=t_eps[:, :])
        nc.vector.reciprocal_approx_fast(t_v[:, :], t_at[:, :])

        # bf16 compact copies
        uvS = sb.tile([128, 1024], bf16, name="uvS")
        nc.scalar.copy(uvS[:, 0:512], t_u[:, :])
        nc.scalar.copy(uvS[:, 512:1024], t_v[:, :])

        # ---------- expand: tile p' -> slot k=p'//8, colblock c=p'%8 ----------
        # dst partition 2k+r gets 8 blocks of 512 (c-major); flat element order
        # of src chunks matches dst [4 parts step 2, 4096].
        uvB = sb.tile([32, NCB * 512], bf16, name="uvB")
        for r in range(2):
            src = uvS[:, r * 512:(r + 1) * 512]
            for q in range(4):
                nc.sync.dma_start(uvB[8 * q + r:8 * q + 8:2, :],
                                  src[32 * q:32 * (q + 1), :])

        # ---------- base GEMM: qb = lb.T @ rb (single bf16 product) ----------
        qb_sb = sb.tile([128, N], f32, name="qb_sb")
        for j in range(4):
            sl = slice(j * 512, (j + 1) * 512)
            qb_ps = ps_base.tile([128, 512], f32, name="qbp")
            nc.tensor.matmul(qb_ps[:, :], t_lb[:, :], t_rb[:, sl],
                             start=True, stop=True)
            nc.scalar.copy(qb_sb[:, sl], qb_ps[:, :])

        # ---------- per-core stats (part A: everything but the qb contractions) ----------
        # ar cols: 0 Sqb, 1 Sqb2, 2 qBsu, 3 qBsv, 4 Su, 5 Sv, 6 Suu, 7 Svv, 8 Suv
        t_ar = sb.tile([128, 12], f32, name="ar_in")
        nc.vector.memset(t_ar[:, :], 0.0)
        t_pack = sb.tile([128, 16], f32, name="pack")
        u3v = t_u[:, :].rearrange("p (j s) -> p j s", s=32)
        v3v = t_v[:, :].rearrange("p (j s) -> p j s", s=32)
        t_su = sb.tile([128, 16], f32, name="su_seg")
        t_sv = sb.tile([128, 16], f32, name="sv_seg")
        nc.vector.tensor_reduce(t_su[:, :], u3v, mybir.AxisListType.X, ALU.add)
        nc.vector.tensor_reduce(t_sv[:, :], v3v, mybir.AxisListType.X, ALU.add)
        nc.vector.tensor_reduce(t_pack[:, 0:1], t_su[:, :], mybir.AxisListType.X, ALU.add)
        nc.vector.tensor_reduce(t_pack[:, 1:2], t_sv[:, :], mybir.AxisListType.X, ALU.add)
        sink_a = sb.tile([128, 512], f32, name="sink_a")
        nc.scalar.activation(sink_a[:, :], t_u[:, :], AF.Square,
                             accum_out=t_pack[:, 2:3])
        nc.scalar.activation(sink_a[:, :], t_v[:, :], AF.Square,
                             accum_out=t_pack[:, 3:4])
        scr = sb.tile([128, 512], f32, name="scr")
        nc.vector.tensor_mul(scr[:, :], t_u[:, :], t_v[:, :])
        nc.scalar.activation(sink_a[:, :], scr[:, :], AF.Copy,
                             accum_out=t_pack[:, 4:5])
        t_ones = sb.tile([128, 128], f32, name="ones")
        nc.vector.memset(t_ones[:, :], 1.0)
        psS = ps_base.tile([128, 8], f32, name="psS", tag="qbp")
        nc.tensor.matmul(psS[:, 0:5], t_ones[:, :], t_pack[:, 0:5],
                         start=True, stop=True)
        nc.scalar.copy(t_ar[:, 4:9], psS[:, 0:5])
        sink_b = sb.tile([128, N], f32, name="sink_b")
        nc.scalar.activation(sink_b[:, :], qb_sb[:, :], AF.Copy,
                             accum_out=t_ar[:, 0:1])
        nc.scalar.activation(sink_b[:, :], qb_sb[:, :], AF.Square,
                             accum_out=t_ar[:, 1:2])
        # su/sv broadcast rows (bf16)
        t_sub = sb.tile([128, 16], bf16, name="su_b")
        t_svb = sb.tile([128, 16], bf16, name="sv_b")
        nc.scalar.copy(t_sub[:, :], t_su[:, :])
        nc.scalar.copy(t_svb[:, :], t_sv[:, :])
        t_rows = sb.tile([1, 2 * N], bf16, name="t_rows")
        t_sur = t_rows[:, 0:N]
        t_svr = t_rows[:, N:2 * N]
        nc.sync.dma_start(t_sur, t_sub[:, :])
        nc.sync.dma_start(t_svr, t_svb[:, :])
        t_one1 = sb.tile([1, 128], bf16, name="ones1")
        nc.vector.memset(t_one1[:, :], 1.0)
        bcS = sb.tile([128, 2 * N], f32, name="bcS")
        scr2 = sb.tile([128, N], f32, name="scr2")

        # ---------- main loop interleaved with stats tail / collective / output ----------
        t_rmax = sb.tile([128, N], f32, name="rmax")
        t_m = sb.tile([128, N], f32, name="t_m")
        t_o = sb.tile([128, N], f32, name="t_o")
        arA = dram.tile([128, 12], f32, name="arA")
        arB = dram.tile([128, 12], f32, name="arB")
        t_arg = sb.tile([128, 12], f32, name="ar_out")
        f1 = sb.tile([128, 12], f32, name="fwork")
        t_epsbn = sb.tile([128, 1], f32, name="epsbn")
        nc.vector.memset(t_epsbn[:, :], BN_EPS)
        t_rs = sb.tile([128, 1], f32, name="rs")
        t_asc = sb.tile([128, 1], f32, name="ascale")
        t_shf = sb.tile([128, 1], f32, name="shift")

        def col(t, i):
            return t[:, i:i + 1]

        def emit_unit(i):
            k, cp = i // 4, i % 4
            wk = t_ws[:, k * 128:(k + 1) * 128]
            psu = ps_main.tile([128, 1024], f32, name="psu")
            for half in range(2):
                cblk = 2 * cp + half
                nc.tensor.matmul(psu[:, half * 512:(half + 1) * 512],
                                 wk, uvB[:, cblk * 512:(cblk + 1) * 512],
                                 start=True, stop=True)
            p0 = 8 * k + 2 * cp
            rdst = t_rmax[:, p0 * 16:p0 * 16 + 32]
            p3 = psu[:, :].rearrange("p (t s) -> p t s", s=32)
            nc.vector.tensor_reduce(rdst, p3, mybir.AxisListType.X, ALU.max)

        def emit_bc_chain():
            for ci, t_row in enumerate((t_sur, t_svr)):
                for j in range(4):
                    sl = slice(j * 512, (j + 1) * 512)
                    bc = ps_tr.tile([128, 512], f32, name="bc")
                    nc.tensor.matmul(bc[:, :], t_one1[:, :], t_row[:, sl],
                                     start=True, stop=True)
                    nc.scalar.copy(bcS[:, ci * N + j * 512:ci * N + (j + 1) * 512],
                                   bc[:, :])

        def emit_qb_contract():
            for ci, c_ in ((0, 2), (1, 3)):
                nc.vector.tensor_mul(scr2[:, :], qb_sb[:, :],
                                     bcS[:, ci * N:(ci + 1) * N])
                nc.scalar.activation(sink_b[:, :], scr2[:, :], AF.Copy,
                                     accum_out=t_ar[:, c_:c_ + 1])
            nc.sync.dma_start(arA[:, :], t_ar[:, :])
            nc.gpsimd.collective_compute(
                "AllReduce", ALU.add,
                replica_groups=[list(range(8))],
                ins=[arA[:, :].opt()],
                outs=[arB[:, :].opt()],
            )
            nc.sync.dma_start(t_arg[:, :], arB[:, :])

        def emit_finalize():
            a_, b_ = col(t_fin, 0), col(t_fin, 1)
            gab, bet = col(t_fin, 2), col(t_fin, 3)
            nc.vector.tensor_scalar_mul(col(f1, 0), col(t_arg, 0), float(S))
            nc.vector.tensor_mul(col(f1, 1), a_, col(t_arg, 4))
            nc.vector.tensor_mul(col(f1, 2), b_, col(t_arg, 5))
            nc.vector.tensor_add(col(f1, 0), col(f1, 0), col(f1, 1))
            nc.vector.tensor_add(col(f1, 0), col(f1, 0), col(f1, 2))
            nc.vector.tensor_scalar_mul(col(f1, 3), col(t_arg, 1), float(S))
            nc.vector.tensor_mul(col(f1, 4), a_, col(t_arg, 2))
            nc.vector.tensor_mul(col(f1, 5), b_, col(t_arg, 3))
            nc.vector.tensor_add(col(f1, 4), col(f1, 4), col(f1, 5))
            nc.vector.tensor_scalar_mul(col(f1, 4), col(f1, 4), 2.0)
            nc.vector.tensor_add(col(f1, 3), col(f1, 3), col(f1, 4))
            nc.vector.tensor_mul(col(f1, 5), a_, a_)
            nc.vector.tensor_mul(col(f1, 5), col(f1, 5), col(t_arg, 6))
            nc.vector.tensor_add(col(f1, 3), col(f1, 3), col(f1, 5))
            nc.vector.tensor_mul(col(f1, 5), b_, b_)
            nc.vector.tensor_mul(col(f1, 5), col(f1, 5), col(t_arg, 7))
            nc.vector.tensor_add(col(f1, 3), col(f1, 3), col(f1, 5))
            nc.vector.tensor_mul(col(f1, 5), a_, b_)
            nc.vector.tensor_mul(col(f1, 5), col(f1, 5), col(t_arg, 8))
            nc.vector.tensor_scalar_mul(col(f1, 5), col(f1, 5), 2.0)
            nc.vector.tensor_add(col(f1, 3), col(f1, 3), col(f1, 5))
            nc.vector.tensor_scalar_mul(col(f1, 6), col(f1, 0), 1.0 / CNT)
            nc.vector.tensor_mul(col(f1, 7), col(f1, 6), col(f1, 6))
            nc.vector.tensor_scalar_mul(col(f1, 8), col(f1, 3), 1.0 / CNT)
            nc.vector.tensor_sub(col(f1, 8), col(f1, 8), col(f1, 7))
            nc.scalar.activation(col(f1, 9), col(f1, 8), AF.Sqrt, bias=t_epsbn[:, :])
            nc.vector.reciprocal(t_rs[:, :], col(f1, 9))
            nc.vector.tensor_mul(t_asc[:, :], gab, t_rs[:, :])
            nc.vector.tensor_mul(t_shf[:, :], col(f1, 6), t_asc[:, :])
            nc.vector.tensor_sub(t_shf[:, :], bet, t_shf[:, :])

        def emit_add(ch):
            sl = slice(ch * 512, (ch + 1) * 512)
            nc.vector.tensor_add(t_m[:, sl], qb_sb[:, sl], t_rmax[:, sl])

        def emit_relu_dma(ch):
            sl = slice(ch * 512, (ch + 1) * 512)
            nc.scalar.activation(t_o[:, sl], t_m[:, sl], AF.Relu,
                                 bias=t_shf[:, :], scale=t_asc[:, :])
            for ci in range(4):
                c = ch * 4 + ci
                csl = slice(c * 128, (c + 1) * 128)
                deng = nc.sync if (c % 2 == 0) else nc.scalar
                deng.dma_start(d_out[:, csl], t_o[:, csl])

        for i in range(2):
            emit_unit(i)
        emit_bc_chain()
        for i in range(2, 12):
            emit_unit(i)
        emit_qb_contract()
        for i in range(12, 32):
            emit_unit(i)
        emit_add(0)
        for i in range(32, 48):
            emit_unit(i)
        emit_add(1)
        emit_add(2)
        for i in range(48, 52):
            emit_unit(i)
        emit_finalize()
        emit_relu_dma(0)
        emit_relu_dma(1)
        emit_relu_dma(2)
        for i in range(52, 64):
            emit_unit(i)
        emit_add(3)
        emit_relu_dma(3)

    nc.compile()
    return nc


def _get_nc():
    if "nc" not in _CACHE:
        _CACHE["nc"] = _build_nc()
    return _CACHE["nc"]


def _prep_inputs(xyz, points, idx, W, b, gamma, beta):
    xyz = np.asarray(xyz, np.float32)
    points = np.asarray(points, np.float32)
    idx = np.asarray(idx).astype(np.int64)
    W = np.asarray(W, np.float32)
    b = np.asarray(b, np.float32)
    gamma = np.asarray(gamma, np.float32)
    beta = np.asarray(beta, np.float32)

    D = points.shape[1]
    q = np.where(gamma >= 0, np.float32(1.0), np.float32(-1.0))
    Wpts = W[:, :D]
    Wu = W[:, D]
    Wc = W[:, D + 1] - Wpts.sum(axis=1)
    Wv = W[:, D + 2]
    lhsb = np.zeros((128, 128), np.float32)
    lhsb[:D, :] = q[None, :] * Wpts.T
    lhsb[126, :] = q * Wc
    lhsb[127, :] = q * b
    lb = lhsb.astype(ml_dtypes.bfloat16)

    a_ = (q * Wu).astype(np.float32)
    b_ = (q * Wv).astype(np.float32)
    ws = np.zeros((32, NSLOT * 128), ml_dtypes.bfloat16)
    for k in range(NSLOT):
        ws[2 * k, k * 128:(k + 1) * 128] = a_.astype(ml_dtypes.bfloat16)
        ws[2 * k + 1, k * 128:(k + 1) * 128] = b_.astype(ml_dtypes.bfloat16)

    fin = np.zeros((128, 8), np.float32)
    fin[:, 0] = a_
    fin[:, 1] = b_
    fin[:, 2] = np.abs(gamma)
    fin[:, 3] = beta

    in_maps = []
    for bb in range(B):
        rhsb = np.concatenate(
            [points[bb], xyz[bb], np.ones((1, N), np.float32)], axis=0)
        g = xyz[bb, 0][idx[bb]]                      # (N, S) host gather
        m = {
            "rb": np.ascontiguousarray(rhsb.astype(ml_dtypes.bfloat16)),
            "lb": lb,
            "gc": np.ascontiguousarray(g.reshape(128, 512).astype(np.float32)),
            "cc": np.ascontiguousarray(xyz[bb].reshape(128, 16)),
            "ws": ws,
            "fin": fin,
        }
        in_maps.append(m)
    return in_maps


def kernel(xyz, points, idx, W, b, gamma, beta, _trace=False):
    from concourse.bass_utils import run_bass_kernel_spmd

    nc = _get_nc()
    in_maps = _prep_inputs(xyz, points, idx, W, b, gamma, beta)
    res = run_bass_kernel_spmd(nc, in_maps, core_ids=list(range(8)),
                               trace=_trace)
    if _trace:
        _CACHE["last_results"] = res
    out = np.stack([res.results[c]["out"] for c in range(8)], axis=0)
    return np.ascontiguousarray(out.transpose(0, 2, 1))


# revision 22
# speedup vs baseline: 1.2349x; 1.0119x over previous
"""PointConvDensity forward on 8 Trainium2 NeuronCores (Bass/Tile).

Math (see reference): per (b, n, s):
    h[o] = W @ feat + bias;  feat = [pts - c, g - 2c, c, 1/(|g-c|+1e-8)]
    BN(train) over (b,n,s) per channel -> relu -> max over s.

Decomposition (rank-2 structure along s):
    h[o,n,s] = qb[o,n] + a[o]*u[n,s] + b[o]*v[n,s]
      qb = lb.T @ [points; xyz; ones]   (K=128 bf16 GEMM, q=sign(gamma) folded)
      u  = g - 2c,  v = 1/(|g-c| + 1e-8),  g = xyz[idx] (host-side layout prep)
    max_s relu(scale*h + shift) = relu(ascale*(qb + max_s(a u + b v)) + shift)
    BN stats from decomposed fp32 sums; one small AllReduce across cores.

Key implementation choices (all validated numerically against the reference;
final rel err ~3.5e-3 vs 2e-2 tolerance):
  - No gpsimd custom-ucode ops: ap_gather / partition_all_reduce trigger
    ~45-225us ucode library reload stalls. The gather is host-side input
    layout prep; the cross-partition stat sum is a ones-matmul on PE.
  - Single bf16 product for the rank-2 term and for the base GEMM.
  - K spread over 32 partitions via 16 weight slots (2 live rows per slot,
    rest zero) so the rhs expand DMA is per-partition balanced.
  - Segmented max = DVE tensor_reduce straight from PSUM (the only engine
    that can do segmented max; bf16 gives no DVE speedup, gpsimd TT is
    rejected by codegen). This is the ~75us critical path.
  - Emission order IS the per-engine schedule: the stats contraction,
    collective, BN finalize and per-chunk output tails are interleaved
    into the 64-unit main loop so nothing head-of-line-blocks DVE and the
    AllReduce latency (~35us) hides under the main loop.
  - Output leaves the device [OUT, N] per core; the host transposes.
"""

import numpy as np
import ml_dtypes

B, N, S = 8, 2048, 32
OUT = 128
BN_EPS = 1e-5
CNT = float(B * N * S)
NSLOT = 16           # weight slots; K = 2*NSLOT = 32
NCB = 8              # column blocks of 512 per slot
NUNIT = 64           # main-loop units (2 tiles / 1024 cols each)

_CACHE = {}


def _build_nc():
    import concourse.bass as bass
    import concourse.bacc as bacc
    import concourse.tile as tile
    import concourse.mybir as mybir
    from contextlib import ExitStack

    f32 = mybir.dt.float32
    bf16 = mybir.dt.bfloat16
    AF = mybir.ActivationFunctionType
    ALU = mybir.AluOpType

    nc = bacc.Bacc("TRN2", target_bir_lowering=False, debug=False, num_devices=8)

    # ---- DRAM I/O (per-core shapes) ----
    d_rb = nc.dram_tensor("rb", [128, N], bf16, kind="ExternalInput").ap()
    d_lb = nc.dram_tensor("lb", [128, 128], bf16, kind="ExternalInput").ap()
    d_gc = nc.dram_tensor("gc", [128, 512], f32, kind="ExternalInput").ap()
    d_cc = nc.dram_tensor("cc", [128, 16], f32, kind="ExternalInput").ap()
    d_ws = nc.dram_tensor("ws", [32, NSLOT * 128], bf16, kind="ExternalInput").ap()
    d_fin = nc.dram_tensor("fin", [128, 8], f32, kind="ExternalInput").ap()
    d_out = nc.dram_tensor("out", [128, N], f32, kind="ExternalOutput").ap()

    with tile.TileContext(nc) as tc, ExitStack() as ctx:
        sb = ctx.enter_context(tc.tile_pool(name="sb", bufs=1))
        ps_base = ctx.enter_context(tc.tile_pool(name="psb", bufs=2, space="PSUM"))
        ps_main = ctx.enter_context(tc.tile_pool(name="psm", bufs=2, space="PSUM"))
        ps_tr = ctx.enter_context(tc.tile_pool(name="pst", bufs=2, space="PSUM"))
        dram = ctx.enter_context(tc.tile_pool(name="dram", bufs=1, space="DRAM"))

        # ---------- input DMAs (gc/cc first: critical path) ----------
        t_gc = sb.tile([128, 512], f32, name="gc")
        t_cc = sb.tile([128, 16], f32, name="cc")
        t_rb = sb.tile([128, N], bf16, name="rb")
        t_lb = sb.tile([128, 128], bf16, name="lb")
        t_ws = sb.tile([32, NSLOT * 128], bf16, name="ws")
        t_fin = sb.tile([128, 8], f32, name="fin")
        nc.sync.dma_start(t_gc[:, 0:256], d_gc[:, 0:256])
        nc.sync.dma_start(t_gc[:, 256:512], d_gc[:, 256:512])
        nc.sync.dma_start(t_cc[:, :], d_cc)
        for j in range(4):
            sl = slice(j * 512, (j + 1) * 512)
            nc.sync.dma_start(t_rb[:, sl], d_rb[:, sl])
        nc.sync.dma_start(t_lb[:, :], d_lb)
        nc.sync.dma_start(t_ws[:, 0:1024], d_ws[:, 0:1024])
        nc.sync.dma_start(t_ws[:, 1024:2048], d_ws[:, 1024:2048])
        nc.sync.dma_start(t_fin[:, :], d_fin)

        # ---------- u, v on the compact layout (partition = 16-n tile) ----------
        cc_b = t_cc[:, :].unsqueeze(2).broadcast_to([128, 16, 32])
        gc3 = t_gc[:, :].rearrange("p (j s) -> p j s", s=32)
        t_t = sb.tile([128, 512], f32, name="t_t")
        t_u = sb.tile([128, 512], f32, name="t_u")
        t_v = sb.tile([128, 512], f32, name="t_v")
        t3 = t_t[:, :].rearrange("p (j s) -> p j s", s=32)
        nc.vector.tensor_sub(t3, gc3, cc_b)
        nc.vector.tensor_sub(t_u[:, :].rearrange("p (j s) -> p j s", s=32), t3, cc_b)
        t_eps = sb.tile([128, 1], f32, name="eps8")
        nc.vector.memset(t_eps[:, :], 1e-8)
        t_at = sb.tile([128, 512], f32, name="t_at")
        nc.scalar.activation(t_at[:, :], t_t[:, :], AF.Abs)
        nc.scalar.activation(t_at[:, :], t_at[:, :], AF.Identity, bias=t_eps[:, :])
        nc.vector.reciprocal_approx_fast(t_v[:, :], t_at[:, :])

        # bf16 compact copies
        uvS = sb.tile([128, 1024], bf16, name="uvS")
        nc.scalar.copy(uvS[:, 0:512], t_u[:, :])
        nc.scalar.copy(uvS[:, 512:1024], t_v[:, :])

        # ---------- expand: tile p' -> slot k=p'//8, colblock c=p'%8 ----------
        # dst partition 2k+r gets 8 blocks of 512 (c-major); flat element order
        # of src chunks matches dst [4 parts step 2, 4096].
        uvB = sb.tile([32, NCB * 512], bf16, name="uvB")
        for r in range(2):
            src = uvS[:, r * 512:(r + 1) * 512]
            for q in range(4):
                nc.sync.dma_start(uvB[8 * q + r:8 * q + 8:2, :],
                                  src[32 * q:32 * (q + 1), :])

        # ---------- base GEMM: qb = lb.T @ rb (single bf16 product) ----------
        qb_sb = sb.tile([128, N], f32, name="qb_sb")
        for j in range(4):
            sl = slice(j * 512, (j + 1) * 512)
            qb_ps = ps_base.tile([128, 512], f32, name="qbp")
            nc.tensor.matmul(qb_ps[:, :], t_lb[:, :], t_rb[:, sl],
                             start=True, stop=True)
            nc.scalar.copy(qb_sb[:, sl], qb_ps[:, :])

        # ---------- per-core stats (part A: everything but the qb contractions) ----------
        # ar cols: 0 Sqb, 1 Sqb2, 2 qBsu, 3 qBsv, 4 Su, 5 Sv, 6 Suu, 7 Svv, 8 Suv
        t_ar = sb.tile([128, 12], f32, name="ar_in")
        nc.vector.memset(t_ar[:, :], 0.0)
        t_pack = sb.tile([128, 16], f32, name="pack")
        u3v = t_u[:, :].rearrange("p (j s) -> p j s", s=32)
        v3v = t_v[:, :].rearrange("p (j s) -> p j s", s=32)
        t_su = sb.tile([128, 16], f32, name="su_seg")
        t_sv = sb.tile([128, 16], f32, name="sv_seg")
        nc.vector.tensor_reduce(t_su[:, :], u3v, mybir.AxisListType.X, ALU.add)
        nc.vector.tensor_reduce(t_sv[:, :], v3v, mybir.AxisListType.X, ALU.add)
        nc.vector.tensor_reduce(t_pack[:, 0:1], t_su[:, :], mybir.AxisListType.X, ALU.add)
        nc.vector.tensor_reduce(t_pack[:, 1:2], t_sv[:, :], mybir.AxisListType.X, ALU.add)
        sink_a = sb.tile([128, 512], f32, name="sink_a")
        nc.scalar.activation(sink_a[:, :], t_u[:, :], AF.Square,
                             accum_out=t_pack[:, 2:3])
        nc.scalar.activation(sink_a[:, :], t_v[:, :], AF.Square,
                             accum_out=t_pack[:, 3:4])
        scr = sb.tile([128, 512], f32, name="scr")
        nc.vector.tensor_mul(scr[:, :], t_u[:, :], t_v[:, :])
        nc.scalar.activation(sink_a[:, :], scr[:, :], AF.Copy,
                             accum_out=t_pack[:, 4:5])
        t_ones = sb.tile([128, 128], f32, name="ones")
        nc.vector.memset(t_ones[:, :], 1.0)
        psS = ps_base.tile([128, 8], f32, name="psS", tag="qbp")
        nc.tensor.matmul(psS[:, 0:5], t_ones[:, :], t_pack[:, 0:5],
                         start=True, stop=True)
        nc.scalar.copy(t_ar[:, 4:9], psS[:, 0:5])
        sink_b = sb.tile([128, N], f32, name="sink_b")
        nc.scalar.activation(sink_b[:, :], qb_sb[:, :], AF.Copy,
                             accum_out=t_ar[:, 0:1])
        nc.scalar.activation(sink_b[:, :], qb_sb[:, :], AF.Square,
                             accum_out=t_ar[:, 1:2])
        # su/sv broadcast rows (bf16)
        t_sub = sb.tile([128, 16], bf16, name="su_b")
        t_svb = sb.tile([128, 16], bf16, name="sv_b")
        nc.scalar.copy(t_sub[:, :], t_su[:, :])
        nc.scalar.copy(t_svb[:, :], t_sv[:, :])
        t_rows = sb.tile([1, 2 * N], bf16, name="t_rows")
        t_sur = t_rows[:, 0:N]
        t_svr = t_rows[:, N:2 * N]
        nc.sync.dma_start(t_sur, t_sub[:, :])
        nc.sync.dma_start(t_svr, t_svb[:, :])
        t_one1 = sb.tile([1, 128], bf16, name="ones1")
        nc.vector.memset(t_one1[:, :], 1.0)
        bcS = sb.tile([128, 2 * N], f32, name="bcS")
        scr2 = sb.tile([128, N], f32, name="scr2")

        # ---------- main loop interleaved with stats tail / collective / output ----------
        t_rmax = sb.tile([128, N], f32, name="rmax")
        t_m = sb.tile([128, N], f32, name="t_m")
        t_o = sb.tile([128, N], f32, name="t_o")
        arA = dram.tile([128, 12], f32, name="arA")
        arB = dram.tile([128, 12], f32, name="arB")
        t_arg = sb.tile([128, 12], f32, name="ar_out")
        f1 = sb.tile([128, 12], f32, name="fwork")
        t_epsbn = sb.tile([128, 1], f32, name="epsbn")
        nc.vector.memset(t_epsbn[:, :], BN_EPS)
        t_rs = sb.tile([128, 1], f32, name="rs")
        t_asc = sb.tile([128, 1], f32, name="ascale")
        t_shf = sb.tile([128, 1], f32, name="shift")

        def col(t, i):
            return t[:, i:i + 1]

        def emit_unit(i):
            k, cp = i // 4, i % 4
            wk = t_ws[:, k * 128:(k + 1) * 128]
            psu = ps_main.tile([128, 1024], f32, name="psu")
            for half in range(2):
                cblk = 2 * cp + half
                nc.tensor.matmul(psu[:, half * 512:(half + 1) * 512],
                                 wk, uvB[:, cblk * 512:(cblk + 1) * 512],
                                 start=True, stop=True)
            p0 = 8 * k + 2 * cp
            rdst = t_rmax[:, p0 * 16:p0 * 16 + 32]
            p3 = psu[:, :].rearrange("p (t s) -> p t s", s=32)
            nc.vector.tensor_reduce(rdst, p3, mybir.AxisListType.X, ALU.max)

        def emit_bc_chain():
            for ci, t_row in enumerate((t_sur, t_svr)):
                for j in range(4):
                    sl = slice(j * 512, (j + 1) * 512)
                    bc = ps_tr.tile([128, 512], f32, name="bc")
                    nc.tensor.matmul(bc[:, :], t_one1[:, :], t_row[:, sl],
                                     start=True, stop=True)
                    nc.scalar.copy(bcS[:, ci * N + j * 512:ci * N + (j + 1) * 512],
                                   bc[:, :])

        def emit_qb_contract():
            for ci, c_ in ((0, 2), (1, 3)):
                nc.vector.tensor_mul(scr2[:, :], qb_sb[:, :],
                                     bcS[:, ci * N:(ci + 1) * N])
                nc.scalar.activation(sink_b[:, :], scr2[:, :], AF.Copy,
                                     accum_out=t_ar[:, c_:c_ + 1])
            nc.sync.dma_start(arA[:, :], t_ar[:, :])
            nc.gpsimd.collective_compute(
                "AllReduce", ALU.add,
                replica_groups=[list(range(8))],
                ins=[arA[:, :].opt()],
                outs=[arB[:, :].opt()],
            )
            nc.sync.dma_start(t_arg[:, :], arB[:, :])

        def emit_finalize():
            a_, b_ = col(t_fin, 0), col(t_fin, 1)
            gab, bet = col(t_fin, 2), col(t_fin, 3)
            nc.vector.tensor_scalar_mul(col(f1, 0), col(t_arg, 0), float(S))
            nc.vector.tensor_mul(col(f1, 1), a_, col(t_arg, 4))
            nc.vector.tensor_mul(col(f1, 2), b_, col(t_arg, 5))
            nc.vector.tensor_add(col(f1, 0), col(f1, 0), col(f1, 1))
            nc.vector.tensor_add(col(f1, 0), col(f1, 0), col(f1, 2))
            nc.vector.tensor_scalar_mul(col(f1, 3), col(t_arg, 1), float(S))
            nc.vector.tensor_mul(col(f1, 4), a_, col(t_arg, 2))
            nc.vector.tensor_mul(col(f1, 5), b_, col(t_arg, 3))
            nc.vector.tensor_add(col(f1, 4), col(f1, 4), col(f1, 5))
            nc.vector.tensor_scalar_mul(col(f1, 4), col(f1, 4), 2.0)
            nc.vector.tensor_add(col(f1, 3), col(f1, 3), col(f1, 4))
            nc.vector.tensor_mul(col(f1, 5), a_, a_)
            nc.vector.tensor_mul(col(f1, 5), col(f1, 5), col(t_arg, 6))
            nc.vector.tensor_add(col(f1, 3), col(f1, 3), col(f1, 5))
            nc.vector.tensor_mul(col(f1, 5), b_, b_)
            nc.vector.tensor_mul(col(f1, 5), col(f1, 5), col(t_arg, 7))
            nc.vector.tensor_add(col(f1, 3), col(f1, 3), col(f1, 5))
            nc.vector.tensor_mul(col(f1, 5), a_, b_)
            nc.vector.tensor_mul(col(f1, 5), col(f1, 5), col(t_arg, 8))
            nc.vector.tensor_scalar_mul(col(f1, 5), col(f1, 5), 2.0)
            nc.vector.tensor_add(col(f1, 3), col(f1, 3), col(f1, 5))
            nc.vector.tensor_scalar_mul(col(f1, 6), col(f1, 0), 1.0 / CNT)
            nc.vector.tensor_mul(col(f1, 7), col(f1, 6), col(f1, 6))
            nc.vector.tensor_scalar_mul(col(f1, 8), col(f1, 3), 1.0 / CNT)
            nc.vector.tensor_sub(col(f1, 8), col(f1, 8), col(f1, 7))
            nc.scalar.activation(col(f1, 9), col(f1, 8), AF.Sqrt, bias=t_epsbn[:, :])
            nc.vector.reciprocal(t_rs[:, :], col(f1, 9))
            nc.vector.tensor_mul(t_asc[:, :], gab, t_rs[:, :])
            nc.vector.tensor_mul(t_shf[:, :], col(f1, 6), t_asc[:, :])
            nc.vector.tensor_sub(t_shf[:, :], bet, t_shf[:, :])

        def emit_add(ch):
            sl = slice(ch * 512, (ch + 1) * 512)
            nc.vector.tensor_add(t_m[:, sl], qb_sb[:, sl], t_rmax[:, sl])

        def emit_relu_dma(ch):
            sl = slice(ch * 512, (ch + 1) * 512)
            nc.scalar.activation(t_o[:, sl], t_m[:, sl], AF.Relu,
                                 bias=t_shf[:, :], scale=t_asc[:, :])
            # 4 partition-quarter DMAs: contiguous 2KB bursts in DRAM rows
            for pi in range(4):
                psl = slice(pi * 32, (pi + 1) * 32)
                deng = nc.sync if (pi % 2 == 0) else nc.scalar
                deng.dma_start(d_out[psl, sl], t_o[psl, sl])

        for i in range(4):
            emit_unit(i)
        emit_bc_chain()
        for i in range(4, 12):
            emit_unit(i)
        emit_qb_contract()
        for i in range(12, 32):
            emit_unit(i)
        emit_add(0)
        for i in range(32, 48):
            emit_unit(i)
        emit_add(1)
        emit_add(2)
        for i in range(48, 52):
            emit_unit(i)
        emit_finalize()
        emit_relu_dma(0)
        emit_relu_dma(1)
        emit_relu_dma(2)
        for i in range(52, 64):
            emit_unit(i)
        emit_add(3)
        emit_relu_dma(3)

    nc.compile()
    return nc


def _get_nc():
    if "nc" not in _CACHE:
        _CACHE["nc"] = _build_nc()
    return _CACHE["nc"]


def _prep_inputs(xyz, points, idx, W, b, gamma, beta):
    xyz = np.asarray(xyz, np.float32)
    points = np.asarray(points, np.float32)
    idx = np.asarray(idx).astype(np.int64)
    W = np.asarray(W, np.float32)
    b = np.asarray(b, np.float32)
    gamma = np.asarray(gamma, np.float32)
    beta = np.asarray(beta, np.float32)

    D = points.shape[1]
    q = np.where(gamma >= 0, np.float32(1.0), np.float32(-1.0))
    Wpts = W[:, :D]
    Wu = W[:, D]
    Wc = W[:, D + 1] - Wpts.sum(axis=1)
    Wv = W[:, D + 2]
    lhsb = np.zeros((128, 128), np.float32)
    lhsb[:D, :] = q[None, :] * Wpts.T
    lhsb[126, :] = q * Wc
    lhsb[127, :] = q * b
    lb = lhsb.astype(ml_dtypes.bfloat16)

    a_ = (q * Wu).astype(np.float32)
    b_ = (q * Wv).astype(np.float32)
    ws = np.zeros((32, NSLOT * 128), ml_dtypes.bfloat16)
    for k in range(NSLOT):
        ws[2 * k, k * 128:(k + 1) * 128] = a_.astype(ml_dtypes.bfloat16)
        ws[2 * k + 1, k * 128:(k + 1) * 128] = b_.astype(ml_dtypes.bfloat16)

    fin = np.zeros((128, 8), np.float32)
    fin[:, 0] = a_
    fin[:, 1] = b_
    fin[:, 2] = np.abs(gamma)
    fin[:, 3] = beta

    in_maps = []
    for bb in range(B):
        rhsb = np.concatenate(
            [points[bb], xyz[bb], np.ones((1, N), np.float32)], axis=0)
        g = xyz[bb, 0][idx[bb]]                      # (N, S) host gather
        m = {
            "rb": np.ascontiguousarray(rhsb.astype(ml_dtypes.bfloat16)),
            "lb": lb,
            "gc": np.ascontiguousarray(g.reshape(128, 512).astype(np.float32)),
            "cc": np.ascontiguousarray(xyz[bb].reshape(128, 16)),
            "ws": ws,
            "fin": fin,
        }
        in_maps.append(m)
    return in_maps


def kernel(xyz, points, idx, W, b, gamma, beta, _trace=False):
    from concourse.bass_utils import run_bass_kernel_spmd

    nc = _get_nc()
    in_maps = _prep_inputs(xyz, points, idx, W, b, gamma, beta)
    res = run_bass_kernel_spmd(nc, in_maps, core_ids=list(range(8)),
                               trace=_trace)
    if _trace:
        _CACHE["last_results"] = res
    out = np.stack([res.results[c]["out"] for c in range(8)], axis=0)
    return np.ascontiguousarray(out.transpose(0, 2, 1))


# revision 25
# speedup vs baseline: 1.3189x; 1.0680x over previous
"""PointConvDensity forward on 8 Trainium2 NeuronCores (Bass/Tile).

Math (see reference): per (b, n, s):
    h[o] = W @ feat + bias;  feat = [pts - c, g - 2c, c, 1/(|g-c|+1e-8)]
    BN(train) over (b,n,s) per channel -> relu -> max over s.

Decomposition (rank-2 structure along s):
    h[o,n,s] = qb[o,n] + a[o]*u[n,s] + b[o]*v[n,s]
      qb = lb.T @ [points; xyz; ones]   (K=128 bf16 GEMM, q=sign(gamma) folded)
      u  = g - 2c,  v = 1/(|g-c| + 1e-8),  g = xyz[idx] (host-side layout prep)
    max_s relu(scale*h + shift) = relu(ascale*(qb + max_s(a u + b v)) + shift)
    BN stats from decomposed fp32 sums; one small AllReduce across cores.

Key implementation choices (all validated numerically against the reference;
final rel err ~3.5e-3 vs 2e-2 tolerance):
  - No gpsimd custom-ucode ops: ap_gather / partition_all_reduce trigger
    ~45-225us ucode library reload stalls. The gather is host-side input
    layout prep; the cross-partition stat sum is a ones-matmul on PE.
  - Single bf16 product for the rank-2 term and for the base GEMM.
  - K spread over 32 partitions via 16 weight slots (2 live rows per slot,
    rest zero) so the rhs expand DMA is per-partition balanced.
  - Segmented max = DVE tensor_reduce straight from PSUM (the only engine
    that can do segmented max; bf16 gives no DVE speedup, gpsimd TT is
    rejected by codegen). This is the ~75us critical path.
  - Emission order IS the per-engine schedule: the stats contraction,
    collective, BN finalize and per-chunk output tails are interleaved
    into the 64-unit main loop so nothing head-of-line-blocks DVE and the
    AllReduce latency (~35us) hides under the main loop.
  - Output leaves the device [OUT, N] per core; the host transposes.
"""

import numpy as np
import ml_dtypes

B, N, S = 8, 2048, 32
OUT = 128
BN_EPS = 1e-5
CNT = float(B * N * S)
NSLOT = 16           # weight slots; K = 2*NSLOT = 32
NCB = 8              # column blocks of 512 per slot
NUNIT = 64           # main-loop units (2 tiles / 1024 cols each)

_CACHE = {}


def _build_nc():
    import concourse.bass as bass
    import concourse.bacc as bacc
    import concourse.tile as tile
    import concourse.mybir as mybir
    from contextlib import ExitStack

    f32 = mybir.dt.float32
    bf16 = mybir.dt.bfloat16
    AF = mybir.ActivationFunctionType
    ALU = mybir.AluOpType

    nc = bacc.Bacc("TRN2", target_bir_lowering=False, debug=False, num_devices=8)

    # ---- DRAM I/O (per-core shapes) ----
    d_rb = nc.dram_tensor("rb", [128, N], bf16, kind="ExternalInput").ap()
    d_lb = nc.dram_tensor("lb", [128, 128], bf16, kind="ExternalInput").ap()
    d_gc = nc.dram_tensor("gc", [128, 512], f32, kind="ExternalInput").ap()
    d_cc = nc.dram_tensor("cc", [128, 16], f32, kind="ExternalInput").ap()
    d_ws = nc.dram_tensor("ws", [32, NSLOT * 128], bf16, kind="ExternalInput").ap()
    d_fin = nc.dram_tensor("fin", [128, 16], f32, kind="ExternalInput").ap()
    d_out = nc.dram_tensor("out", [128, N], f32, kind="ExternalOutput").ap()

    with tile.TileContext(nc) as tc, ExitStack() as ctx:
        sb = ctx.enter_context(tc.tile_pool(name="sb", bufs=1))
        ps_main = ctx.enter_context(tc.tile_pool(name="psm", bufs=2, space="PSUM"))
        dram = ctx.enter_context(tc.tile_pool(name="dram", bufs=1, space="DRAM"))

        # ---------- input DMAs (gc/cc first: critical path) ----------
        t_gc = sb.tile([128, 512], f32, name="gc")
        t_cc = sb.tile([128, 16], f32, name="cc")
        t_rb = sb.tile([128, N], bf16, name="rb")
        t_lb = sb.tile([128, 128], bf16, name="lb")
        t_ws = sb.tile([32, NSLOT * 128], bf16, name="ws")
        t_fin = sb.tile([128, 16], f32, name="fin")
        nc.sync.dma_start(t_gc[:, 0:256], d_gc[:, 0:256])
        nc.sync.dma_start(t_gc[:, 256:512], d_gc[:, 256:512])
        nc.sync.dma_start(t_cc[:, :], d_cc)
        for j in range(4):
            sl = slice(j * 512, (j + 1) * 512)
            nc.sync.dma_start(t_rb[:, sl], d_rb[:, sl])
        nc.sync.dma_start(t_lb[:, :], d_lb)
        nc.sync.dma_start(t_ws[:, 0:1024], d_ws[:, 0:1024])
        nc.sync.dma_start(t_ws[:, 1024:2048], d_ws[:, 1024:2048])
        nc.sync.dma_start(t_fin[:, :], d_fin)

        # ---------- u, v on the compact layout (partition = 16-n tile) ----------
        cc_b = t_cc[:, :].unsqueeze(2).broadcast_to([128, 16, 32])
        gc3 = t_gc[:, :].rearrange("p (j s) -> p j s", s=32)
        t_t = sb.tile([128, 512], f32, name="t_t")
        t_u = sb.tile([128, 512], f32, name="t_u")
        t_v = sb.tile([128, 512], f32, name="t_v")
        t3 = t_t[:, :].rearrange("p (j s) -> p j s", s=32)
        nc.vector.tensor_sub(t3, gc3, cc_b)
        nc.vector.tensor_sub(t_u[:, :].rearrange("p (j s) -> p j s", s=32), t3, cc_b)
        t_eps = sb.tile([128, 1], f32, name="eps8")
        nc.vector.memset(t_eps[:, :], 1e-8)
        t_at = sb.tile([128, 512], f32, name="t_at")
        nc.scalar.activation(t_at[:, :], t_t[:, :], AF.Abs)
        nc.scalar.activation(t_at[:, :], t_at[:, :], AF.Identity, bias=t_eps[:, :])
        nc.vector.reciprocal_approx_fast(t_v[:, :], t_at[:, :])

        # bf16 compact copies
        uvS = sb.tile([128, 1024], bf16, name="uvS")
        nc.scalar.copy(uvS[:, 0:512], t_u[:, :])
        nc.scalar.copy(uvS[:, 512:1024], t_v[:, :])

        # ---------- expand: tile p' -> slot k=p'//8, colblock c=p'%8 ----------
        # dst partition 2k+r gets 8 blocks of 512 (c-major); flat element order
        # of src chunks matches dst [4 parts step 2, 4096].
        uvB = sb.tile([32, NCB * 512], bf16, name="uvB")
        for r in range(2):
            src = uvS[:, r * 512:(r + 1) * 512]
            for q in range(4):
                nc.sync.dma_start(uvB[8 * q + r:8 * q + 8:2, :],
                                  src[32 * q:32 * (q + 1), :])

        # ---------- base GEMM: qb = lb.T @ rb (single bf16 product) ----------
        qb_sb = sb.tile([128, N], f32, name="qb_sb")
        qb_ps = ps_main.tile([128, 2048], f32, name="psu")
        for j in range(4):
            sl = slice(j * 512, (j + 1) * 512)
            nc.tensor.matmul(qb_ps[:, sl], t_lb[:, :], t_rb[:, sl],
                             start=True, stop=True)
        nc.scalar.copy(qb_sb[:, 0:1024], qb_ps[:, 0:1024])
        nc.scalar.copy(qb_sb[:, 1024:2048], qb_ps[:, 1024:2048])

        # ---------- per-core stats (part A: everything but the qb contractions) ----------
        # ar cols: 0 Sqb, 1 Sqb2, 2 qBsu, 3 qBsv, 4 Su, 5 Sv, 6 Suu, 7 Svv, 8 Suv
        t_ar = sb.tile([128, 12], f32, name="ar_in")
        nc.vector.memset(t_ar[:, :], 0.0)
        t_pack = sb.tile([128, 16], f32, name="pack")
        u3v = t_u[:, :].rearrange("p (j s) -> p j s", s=32)
        v3v = t_v[:, :].rearrange("p (j s) -> p j s", s=32)
        t_su = sb.tile([128, 16], f32, name="su_seg")
        t_sv = sb.tile([128, 16], f32, name="sv_seg")
        nc.vector.tensor_reduce(t_su[:, :], u3v, mybir.AxisListType.X, ALU.add)
        nc.vector.tensor_reduce(t_sv[:, :], v3v, mybir.AxisListType.X, ALU.add)
        nc.vector.tensor_reduce(t_pack[:, 0:1], t_su[:, :], mybir.AxisListType.X, ALU.add)
        nc.vector.tensor_reduce(t_pack[:, 1:2], t_sv[:, :], mybir.AxisListType.X, ALU.add)
        sink_a = sb.tile([128, 512], f32, name="sink_a")
        nc.scalar.activation(sink_a[:, :], t_u[:, :], AF.Square,
                             accum_out=t_pack[:, 2:3])
        nc.scalar.activation(sink_a[:, :], t_v[:, :], AF.Square,
                             accum_out=t_pack[:, 3:4])
        scr = sb.tile([128, 512], f32, name="scr")
        nc.vector.tensor_mul(scr[:, :], t_u[:, :], t_v[:, :])
        nc.scalar.activation(sink_a[:, :], scr[:, :], AF.Copy,
                             accum_out=t_pack[:, 4:5])
        t_ones = sb.tile([128, 128], f32, name="ones")
        nc.vector.memset(t_ones[:, :], 1.0)

        def emit_psS():
            psS = ps_main.tile([128, 8], f32, name="psS", tag="psu")
            nc.tensor.matmul(psS[:, 0:5], t_ones[:, :], t_pack[:, 0:5],
                             start=True, stop=True)
            nc.scalar.copy(t_ar[:, 1:3], psS[:, 0:2])
            nc.scalar.copy(t_ar[:, 6:9], psS[:, 2:5])

        sink_b = sb.tile([128, N], f32, name="sink_b")
        # su/sv broadcast rows (bf16)
        t_sub = sb.tile([128, 16], bf16, name="su_b")
        t_svb = sb.tile([128, 16], bf16, name="sv_b")
        nc.scalar.copy(t_sub[:, :], t_su[:, :])
        nc.scalar.copy(t_svb[:, :], t_sv[:, :])
        t_rows = sb.tile([1, 2 * N], bf16, name="t_rows")
        t_sur = t_rows[:, 0:N]
        t_svr = t_rows[:, N:2 * N]
        nc.sync.dma_start(t_sur, t_sub[:, :])
        nc.sync.dma_start(t_svr, t_svb[:, :])
        t_one1 = sb.tile([1, 128], bf16, name="ones1")
        nc.vector.memset(t_one1[:, :], 1.0)
        bcS = sb.tile([128, 2 * N], f32, name="bcS")
        scr2 = sb.tile([128, N], f32, name="scr2")

        # ---------- main loop interleaved with stats tail / collective / output ----------
        t_rmax = sb.tile([128, N], f32, name="rmax")
        t_m = sb.tile([128, N], f32, name="t_m")
        t_o = sb.tile([128, N], f32, name="t_o")
        arA = dram.tile([128, 12], f32, name="arA")
        arB = dram.tile([128, 12], f32, name="arB")
        t_arg = sb.tile([128, 12], f32, name="ar_out")
        f1 = sb.tile([128, 12], f32, name="fwork")
        t_epsbn = sb.tile([128, 1], f32, name="epsbn")
        nc.vector.memset(t_epsbn[:, :], BN_EPS)
        t_rs = sb.tile([128, 1], f32, name="rs")
        t_asc = sb.tile([128, 1], f32, name="ascale")
        t_shf = sb.tile([128, 1], f32, name="shift")

        def col(t, i):
            return t[:, i:i + 1]

        def emit_unit(i):
            # big unit i in 0..31: slot k = i//2, col-half cp2 = i%2
            k, cp2 = i // 2, i % 2
            wk = t_ws[:, k * 128:(k + 1) * 128]
            psu = ps_main.tile([128, 2048], f32, name="psu")
            for q in range(4):
                cblk = 4 * cp2 + q
                nc.tensor.matmul(psu[:, q * 512:(q + 1) * 512],
                                 wk, uvB[:, cblk * 512:(cblk + 1) * 512],
                                 start=True, stop=True)
            p0 = 8 * k + 4 * cp2
            rdst = t_rmax[:, p0 * 16:p0 * 16 + 64]
            p3 = psu[:, :].rearrange("p (t s) -> p t s", s=32)
            nc.vector.tensor_reduce(rdst, p3, mybir.AxisListType.X, ALU.max)

        def emit_bc_chain():
            for ci, t_row in enumerate((t_sur, t_svr)):
                bc = ps_main.tile([128, 2048], f32, name="bcp", tag="psu")
                for j in range(4):
                    sl = slice(j * 512, (j + 1) * 512)
                    nc.tensor.matmul(bc[:, sl], t_one1[:, :], t_row[:, sl],
                                     start=True, stop=True)
                nc.scalar.copy(bcS[:, ci * N:(ci + 1) * N], bc[:, :])

        def emit_qb_contract():
            nc.scalar.activation(sink_b[:, :], qb_sb[:, :], AF.Copy,
                                 accum_out=t_ar[:, 0:1])
            nc.scalar.activation(sink_b[:, :], qb_sb[:, :], AF.Square,
                                 accum_out=t_ar[:, 3:4])
            for ci, c_ in ((0, 4), (1, 5)):
                nc.vector.tensor_mul(scr2[:, :], qb_sb[:, :],
                                     bcS[:, ci * N:(ci + 1) * N])
                nc.scalar.activation(sink_b[:, :], scr2[:, :], AF.Copy,
                                     accum_out=t_ar[:, c_:c_ + 1])
            nc.sync.dma_start(arA[:, :], t_ar[:, :])
            nc.gpsimd.collective_compute(
                "AllReduce", ALU.add,
                replica_groups=[list(range(8))],
                ins=[arA[:, :].opt()],
                outs=[arB[:, :].opt()],
            )
            nc.sync.dma_start(t_arg[:, :], arB[:, :])

        def emit_finalize():
            # coef-packed: f1[0:9] = t_arg[0:9] * cf[0:9];
            # Sh = sum(f1[0:3]), Sh2 = sum(f1[3:9])
            gab, bet = col(t_fin, 2), col(t_fin, 3)
            nc.vector.tensor_mul(f1[:, 0:9], t_arg[:, 0:9], t_fin[:, 4:13])
            nc.vector.tensor_reduce(col(f1, 9), f1[:, 0:3],
                                    mybir.AxisListType.X, ALU.add)
            nc.vector.tensor_reduce(col(f1, 10), f1[:, 3:9],
                                    mybir.AxisListType.X, ALU.add)
            nc.vector.tensor_scalar_mul(col(f1, 9), col(f1, 9), 1.0 / CNT)
            nc.vector.tensor_scalar_mul(col(f1, 10), col(f1, 10), 1.0 / CNT)
            nc.vector.tensor_mul(col(f1, 11), col(f1, 9), col(f1, 9))
            nc.vector.tensor_sub(col(f1, 10), col(f1, 10), col(f1, 11))
            nc.scalar.activation(col(f1, 10), col(f1, 10), AF.Sqrt,
                                 bias=t_epsbn[:, :])
            nc.vector.reciprocal(t_rs[:, :], col(f1, 10))
            nc.vector.tensor_mul(t_asc[:, :], gab, t_rs[:, :])
            nc.vector.tensor_mul(t_shf[:, :], col(f1, 9), t_asc[:, :])
            nc.vector.tensor_sub(t_shf[:, :], bet, t_shf[:, :])

        def _chsl(ch, half):
            if half is None:
                return slice(ch * 512, (ch + 1) * 512)
            return slice(ch * 512 + half * 256, ch * 512 + (half + 1) * 256)

        def emit_add(ch, half=None):
            sl = _chsl(ch, half)
            nc.vector.tensor_add(t_m[:, sl], qb_sb[:, sl], t_rmax[:, sl])

        def emit_relu_dma(ch, half=None):
            sl = _chsl(ch, half)
            nc.scalar.activation(t_o[:, sl], t_m[:, sl], AF.Relu,
                                 bias=t_shf[:, :], scale=t_asc[:, :])
            # partition-sliced DMAs: contiguous >=1KB bursts in DRAM rows
            for pi in range(4):
                psl = slice(pi * 32, (pi + 1) * 32)
                deng = nc.sync if (pi % 2 == 0) else nc.scalar
                deng.dma_start(d_out[psl, sl], t_o[psl, sl])

        for i in range(2):
            emit_unit(i)
        emit_psS()
        emit_bc_chain()
        for i in range(2, 6):
            emit_unit(i)
        emit_qb_contract()
        for i in range(6, 16):
            emit_unit(i)
        emit_add(0)
        for i in range(16, 24):
            emit_unit(i)
        emit_add(1)
        emit_add(2)
        emit_finalize()
        emit_relu_dma(0)
        emit_relu_dma(1)
        emit_relu_dma(2)
        for i in range(24, 28):
            emit_unit(i)
        emit_add(3, half=0)
        for i in range(28, 32):
            emit_unit(i)
        emit_add(3, half=1)
        emit_relu_dma(3, half=0)
        emit_relu_dma(3, half=1)

    nc.compile()
    return nc


def _get_nc():
    if "nc" not in _CACHE:
        _CACHE["nc"] = _build_nc()
    return _CACHE["nc"]


def _prep_inputs(xyz, points, idx, W, b, gamma, beta):
    xyz = np.asarray(xyz, np.float32)
    points = np.asarray(points, np.float32)
    idx = np.asarray(idx).astype(np.int64)
    W = np.asarray(W, np.float32)
    b = np.asarray(b, np.float32)
    gamma = np.asarray(gamma, np.float32)
    beta = np.asarray(beta, np.float32)

    D = points.shape[1]
    q = np.where(gamma >= 0, np.float32(1.0), np.float32(-1.0))
    Wpts = W[:, :D]
    Wu = W[:, D]
    Wc = W[:, D + 1] - Wpts.sum(axis=1)
    Wv = W[:, D + 2]
    lhsb = np.zeros((128, 128), np.float32)
    lhsb[:D, :] = q[None, :] * Wpts.T
    lhsb[126, :] = q * Wc
    lhsb[127, :] = q * b
    lb = lhsb.astype(ml_dtypes.bfloat16)

    a_ = (q * Wu).astype(np.float32)
    b_ = (q * Wv).astype(np.float32)
    ws = np.zeros((32, NSLOT * 128), ml_dtypes.bfloat16)
    for k in range(NSLOT):
        ws[2 * k, k * 128:(k + 1) * 128] = a_.astype(ml_dtypes.bfloat16)
        ws[2 * k + 1, k * 128:(k + 1) * 128] = b_.astype(ml_dtypes.bfloat16)

    fin = np.zeros((128, 16), np.float32)
    fin[:, 0] = a_
    fin[:, 1] = b_
    fin[:, 2] = np.abs(gamma)
    fin[:, 3] = beta
    # finalize coef columns (ar layout [Sqb,Su,Sv | Sqb2,qBsu,qBsv,Suu,Svv,Suv])
    fin[:, 4] = float(S)
    fin[:, 5] = a_
    fin[:, 6] = b_
    fin[:, 7] = float(S)
    fin[:, 8] = 2.0 * a_
    fin[:, 9] = 2.0 * b_
    fin[:, 10] = a_ * a_
    fin[:, 11] = b_ * b_
    fin[:, 12] = 2.0 * a_ * b_

    in_maps = []
    for bb in range(B):
        rhsb = np.concatenate(
            [points[bb], xyz[bb], np.ones((1, N), np.float32)], axis=0)
        g = xyz[bb, 0][idx[bb]]                      # (N, S) host gather
        m = {
            "rb": np.ascontiguousarray(rhsb.astype(ml_dtypes.bfloat16)),
            "lb": lb,
            "gc": np.ascontiguousarray(g.reshape(128, 512).astype(np.float32)),
            "cc": np.ascontiguousarray(xyz[bb].reshape(128, 16)),
            "ws": ws,
            "fin": fin,
        }
        in_maps.append(m)
    return in_maps


def kernel(xyz, points, idx, W, b, gamma, beta, _trace=False):
    from concourse.bass_utils import run_bass_kernel_spmd

    nc = _get_nc()
    in_maps = _prep_inputs(xyz, points, idx, W, b, gamma, beta)
    res = run_bass_kernel_spmd(nc, in_maps, core_ids=list(range(8)),
                               trace=_trace)
    if _trace:
        _CACHE["last_results"] = res
    out = np.stack([res.results[c]["out"] for c in range(8)], axis=0)
    return np.ascontiguousarray(out.transpose(0, 2, 1))
